# revision 17
# baseline (speedup 1.0000x reference)
"""Trainium2 Bass kernel for nn_MemoryWeightedAttention.

out[b,h,q,k] = attention_scores[b,h,q,k] * (1 + 0.066 * mod[b,q])

where mod[b,q] is a small LN/MLP pipeline applied to
(log1p(global_count[id]), tanh-MLP(LN(emb[id]))) -- i.e. mod depends ONLY on
the token id at (b,q) plus a global histogram of input_ids.  So we compute the
full pipeline once over the 1024-entry vocabulary table (in transposed layout:
features on partitions, vocab on the free dim -> no transposes between
layers), then gather per-token scales with one-hot matmuls, and stream the
attention_scores tensor (as fp16) through a per-partition scalar multiply.

Sharding: 8 cores, core i handles batch i//4, heads 4*(i%4) .. 4*(i%4)+4
(a contiguous [8192, 2048] row-slice of the flattened [65536, 2048] scores).
The tiny table pipeline is replicated on every core (no collectives).

Precision: the scores stream is fp16 (input quantization ~5e-4 rel) and the
table pipeline runs in bf16 on the PE (4x faster than fp32); the resulting
scale factor (1 + 0.066*tanh(...)) error is ~1e-4.  Total mean rel err
~5e-4, far under the 2e-2 gate.
"""

import sys

for _p in ("/opt/trn_rl_repo",):
    if _p not in sys.path:
        sys.path.insert(0, _p)

from contextlib import ExitStack

import numpy as np
import ml_dtypes

import concourse.bacc as bacc
import concourse.tile as tile
from concourse import mybir
from concourse.bass import ts
from concourse.bass_utils import run_bass_kernel_spmd
from concourse.masks import make_identity

F32 = mybir.dt.float32
F16 = mybir.dt.float16
F8 = mybir.dt.float8e4
BF16 = mybir.dt.bfloat16
AF = mybir.ActivationFunctionType
ALU = mybir.AluOpType

B, H, S = 2, 16, 2048
CTX = 1024
VD = 288          # valence dim
FFN = 512
EPSILON = 0.066
LN_EPS = 1e-5

N_CORES = 8
HEADS_PER_CORE = H * B // N_CORES          # 4
SHARD_ROWS = HEADS_PER_CORE * S            # 8192
N_TOK = B * S                              # 4096 (global, for counts)
S_TILES = S // 128                         # 16 token tiles per core's batch
TOK_TILES = N_TOK // 128                   # 32 global token tiles
CTX_CHUNKS = CTX // 128                    # 8
ROWS_PER_CHUNK = 512                       # score rows per DMA chunk
N_CHUNKS = SHARD_ROWS // ROWS_PER_CHUNK    # 16
SUB_TILES = ROWS_PER_CHUNK // 128          # 4
STREAM_BUFS = 8


def _chunks(n, c=128):
    out = []
    o = 0
    while o < n:
        out.append((o, min(c, n - o)))
        o += c
    return out


def build_nc(mod=True, stream=True, mod_rep=1, stream_rep=1):
    nc = bacc.Bacc("TRN2", target_bir_lowering=False, debug=False,
                   num_devices=N_CORES)

    dt = nc.dram_tensor
    scores = dt("scores", [SHARD_ROWS, S], F16, kind="ExternalInput")
    ids_loc = dt("ids_loc", [1, S], F16, kind="ExternalInput")
    ids_rem = dt("ids_rem", [1, S], F16, kind="ExternalInput")
    iota_cols = dt("iota_cols", [128, CTX_CHUNKS], F32, kind="ExternalInput")
    embT = dt("embT", [VD, CTX], BF16, kind="ExternalInput")
    pW1 = dt("pW1", [VD, 2 * VD], BF16, kind="ExternalInput")
    pb1 = dt("pb1", [128, 5], F32, kind="ExternalInput")
    pW2 = dt("pW2", [2 * VD, H], BF16, kind="ExternalInput")
    pb2 = dt("pb2", [H, 1], F32, kind="ExternalInput")
    fW1 = dt("fW1", [H + 1, FFN], BF16, kind="ExternalInput")
    fb1 = dt("fb1", [128, 4], F32, kind="ExternalInput")
    fW2 = dt("fW2", [FFN, FFN // 2], BF16, kind="ExternalInput")
    fb2 = dt("fb2", [128, 2], F32, kind="ExternalInput")
    fW3 = dt("fW3", [FFN // 2, 1], BF16, kind="ExternalInput")
    fb3 = dt("fb3", [1, 1], F32, kind="ExternalInput")
    lnvw = dt("lnvw", [128, 3], F32, kind="ExternalInput")   # VD cols chunked
    lnvb = dt("lnvb", [128, 3], F32, kind="ExternalInput")
    lncw = dt("lncw", [H + 1, 1], F32, kind="ExternalInput")
    lncb = dt("lncb", [H + 1, 1], F32, kind="ExternalInput")
    out = dt("out", [SHARD_ROWS, S], F16, kind="ExternalOutput")

    VD_CH = _chunks(VD)            # [(0,128),(128,128),(256,32)]
    VD2_CH = _chunks(2 * VD)       # 576 -> 5 chunks
    FFN_CH = _chunks(FFN)          # 512 -> 4
    FFNH_CH = _chunks(FFN // 2)    # 256 -> 2
    NSPLIT = [(0, 512), (512, 512)]   # vocab free-dim split

    with tile.TileContext(nc) as tc, ExitStack() as ctx:
        singles = ctx.enter_context(tc.tile_pool(name="singles", bufs=1))
        work = ctx.enter_context(tc.tile_pool(name="work", bufs=8))
        otpool = ctx.enter_context(tc.tile_pool(name="otpool", bufs=1))
        ps = ctx.enter_context(tc.tile_pool(name="ps", bufs=3, space="PSUM"))
        pcol = ctx.enter_context(tc.tile_pool(name="pcol", bufs=1, space="PSUM"))
        sc_pool = ctx.enter_context(tc.tile_pool(name="sc", bufs=STREAM_BUFS))

        # ---------- constants / weights into SBUF ----------
        _uid = [0]

        def load(shape, src, dtype=F32):
            _uid[0] += 1
            t = singles.tile(shape, dtype, tag=f"s{_uid[0]}", name=f"s{_uid[0]}")
            nc.sync.dma_start(out=t[:], in_=src)
            return t

        def stile(shape, dtype=F32):
            _uid[0] += 1
            return singles.tile(shape, dtype, tag=f"s{_uid[0]}", name=f"s{_uid[0]}")

        embT_sb = []
        for o, sz in VD_CH:
            _t = work.tile([sz, CTX], BF16, tag="work", name=f"embT{o}")
            nc.sync.dma_start(out=_t[:], in_=embT[o:o + sz, :])
            embT_sb.append(_t)
        lnvw_sb = load([128, 3], lnvw[:, :])
        lnvb_sb = load([128, 3], lnvb[:, :])
        iota_cols_sb = load([128, CTX_CHUNKS], iota_cols[:, :], F32)
        ids_loc_b = stile([128, S], F16)
        nc.sync.dma_start(out=ids_loc_b[:], in_=ids_loc.ap().to_broadcast((128, S)))
        ids_rem_b = stile([128, S], F16)
        nc.sync.dma_start(out=ids_rem_b[:], in_=ids_rem.ap().to_broadcast((128, S)))
        pW1_sb = [load([sz, 2 * VD], pW1[o:o + sz, :], BF16) for o, sz in VD_CH]
        pW2_sb = [load([sz, H], pW2[o:o + sz, :], BF16) for o, sz in VD2_CH]
        fW1_sb = load([H + 1, FFN], fW1[:, :], BF16)
        fW2_sb = [load([sz, FFN // 2], fW2[o:o + sz, :], BF16) for o, sz in FFN_CH]
        fW3_sb = [load([sz, 1], fW3[o:o + sz, :], BF16) for o, sz in FFNH_CH]
        pb1_sb = load([128, 5], pb1[:, :])
        pb2_sb = load([H, 1], pb2[:, :])
        fb1_sb = load([128, 4], fb1[:, :])
        fb2_sb = load([128, 2], fb2[:, :])
        fb3_sb = load([1, 1], fb3[:, :])
        lncw_sb = load([H + 1, 1], lncw[:, :])
        lncb_sb = load([H + 1, 1], lncb[:, :])

        ones_col = stile([128, 1], BF16)
        nc.vector.memset(ones_col[:], 1.0)
        ones_row = stile([1, 128], BF16)
        nc.vector.memset(ones_row[:], 1.0)
        identity = stile([128, 128], F32)
        make_identity(nc, identity[:])

        eps_sb = stile([128, 1], F32)
        nc.vector.memset(eps_sb[:], LN_EPS)
        modv16 = stile([128, CTX_CHUNKS], F16)
        scales_sb = stile([128, S_TILES], F32)

        if not mod:
            nc.vector.memset(scales_sb[:], 1.0)
        for _mrep in range(mod_rep if mod else 0):
            # ---------- LN_v of emb table, transposed layout ----------
            # sum / sumsq rows over the 288 feature partitions via ones-lhsT matmul
            sum_ps = ps.tile([1, CTX], F32, tag="ps")
            sumsq_ps = ps.tile([1, CTX], F32, tag="ps")
            sq_t = []
            for k, (o, sz) in enumerate(VD_CH):
                sq = work.tile([sz, CTX], BF16, tag="work")
                nc.scalar.activation(out=sq[:], in_=embT_sb[k][:], func=AF.Square)
                sq_t.append(sq)
            for n0, nsz in NSPLIT:
                for k, (o, sz) in enumerate(VD_CH):
                    nc.tensor.matmul(sum_ps[:1, n0:n0 + nsz], ones_col[:sz, :],
                                     embT_sb[k][:, n0:n0 + nsz],
                                     start=(k == 0), stop=(k == len(VD_CH) - 1))
                for k, (o, sz) in enumerate(VD_CH):
                    nc.tensor.matmul(sumsq_ps[:1, n0:n0 + nsz], ones_col[:sz, :],
                                     sq_t[k][:, n0:n0 + nsz],
                                     start=(k == 0), stop=(k == len(VD_CH) - 1))


            # ---------- LN_v stats rows ----------
            m_row = work.tile([1, CTX], BF16, tag="work")
            nc.scalar.activation(out=m_row[:], in_=sum_ps[:1, :], func=AF.Copy,
                                 scale=1.0 / VD)
            msq_row = work.tile([1, CTX], F32, tag="work")
            nc.scalar.activation(out=msq_row[:], in_=sum_ps[:1, :], func=AF.Square,
                                 scale=1.0 / VD)
            varp = work.tile([1, CTX], F32, tag="work")
            nc.vector.tensor_scalar_mul(varp[:], sumsq_ps[:1, :], 1.0 / VD)
            nc.vector.tensor_tensor(out=varp[:], in0=varp[:], in1=msq_row[:],
                                    op=ALU.subtract)
            varp_bf = work.tile([1, CTX], BF16, tag="work")
            nc.scalar.activation(out=varp_bf[:], in_=varp[:],
                                 func=AF.Abs_reciprocal_sqrt, bias=eps_sb[:1, :])

            # broadcast mean / rstd rows across 128 partitions (bf16 PE matmul)
            mean_bc = ps.tile([128, CTX], F32, tag="ps")
            rstd_bc = ps.tile([128, CTX], F32, tag="ps")
            for n0, nsz in NSPLIT:
                nc.tensor.matmul(mean_bc[:, n0:n0 + nsz], ones_row[:1, :],
                                 m_row[:1, n0:n0 + nsz], start=True, stop=True)
                nc.tensor.matmul(rstd_bc[:, n0:n0 + nsz], ones_row[:1, :],
                                 varp_bf[:1, n0:n0 + nsz], start=True, stop=True)

            # ---------- normalized emb table E'T ----------
            e1t = []
            for k, (o, sz) in enumerate(VD_CH):
                e1 = work.tile([sz, CTX], F32, tag="work")
                nc.vector.tensor_tensor(out=e1[:], in0=embT_sb[k][:],
                                        in1=mean_bc[:sz, :], op=ALU.subtract)
                nc.vector.tensor_tensor(out=e1[:], in0=e1[:], in1=rstd_bc[:sz, :],
                                        op=ALU.mult)
                e1b = work.tile([sz, CTX], BF16, tag="work")
                nc.vector.tensor_scalar(e1b[:], e1[:], lnvw_sb[:sz, k:k + 1],
                                        lnvb_sb[:sz, k:k + 1],
                                        op0=ALU.mult, op1=ALU.add)
                e1t.append(e1b)

            # ---------- global token histogram (counts) ----------
            # One-hot tiles OT[c][p, t] = (ids[t] == c*128+p) serve double
            # duty: their fused accum_out column is this batch's histogram
            # contribution (sum over tokens), and the tiles feed the mod
            # gather matmuls at the end of the pipeline.  The remote batch
            # contributes via accum-only one-hots into a reused scratch tile.
            acc_loc = stile([128, CTX_CHUNKS], F32)
            acc_rem = stile([128, CTX_CHUNKS], F32)
            ot_sb = []
            for c in range(CTX_CHUNKS):
                _o = otpool.tile([128, S], F8, tag=f"ot{c}", name=f"ot{c}")
                nc.gpsimd.tensor_scalar(_o[:], ids_loc_b[:],
                                        iota_cols_sb[:, c:c + 1], None,
                                        op0=ALU.is_equal)
                ot_sb.append(_o)
            for c in range(CTX_CHUNKS):
                nc.scalar.activation(out=ot_sb[c][:], in_=ot_sb[c][:],
                                     func=AF.Copy,
                                     accum_out=acc_loc[:, c:c + 1])
            for c in range(CTX_CHUNKS):
                otr = otpool.tile([128, S], F8, tag="otr", name=f"otr{c}")
                nc.vector.tensor_scalar(otr[:], ids_rem_b[:],
                                        iota_cols_sb[:, c:c + 1], 0.0,
                                        op0=ALU.is_equal, op1=ALU.add,
                                        accum_out=acc_rem[:, c:c + 1])
            counts_cs = stile([128, CTX_CHUNKS], F32)
            nc.vector.tensor_tensor(out=counts_cs[:], in0=acc_loc[:],
                                    in1=acc_rem[:], op=ALU.add)

            # ---------- layer A (gelu(E' @ pW1 + pb1)) fused with ----------
            # ---------- layer B (tanh(H1 @ pW2 + pb2)) accumulation ----------
            combT = work.tile([H + 1, CTX], BF16, tag="work")
            val_ps = ps.tile([H, CTX], F32, tag="ps")
            for m, (mo, msz) in enumerate(VD2_CH):
                h1_ps = ps.tile([msz, CTX], F32, tag="ps")
                for n0, nsz in NSPLIT:
                    for k, (o, sz) in enumerate(VD_CH):
                        nc.tensor.matmul(h1_ps[:, n0:n0 + nsz],
                                         pW1_sb[k][:, mo:mo + msz],
                                         e1t[k][:, n0:n0 + nsz],
                                         start=(k == 0),
                                         stop=(k == len(VD_CH) - 1))
                h1 = work.tile([msz, CTX], BF16, tag="work")
                nc.scalar.activation(out=h1[:], in_=h1_ps[:], func=AF.Gelu,
                                     bias=pb1_sb[:msz, m:m + 1])
                for n0, nsz in NSPLIT:
                    nc.tensor.matmul(val_ps[:, n0:n0 + nsz], pW2_sb[m][:, :],
                                     h1[:, n0:n0 + nsz],
                                     start=(m == 0), stop=(m == len(VD2_CH) - 1))

            counts_ps = ps.tile([1, CTX], F32, tag="ps")
            for c in range(CTX_CHUNKS):
                nc.tensor.transpose(counts_ps[:1, ts(c, 128)],
                                    counts_cs[:, c:c + 1], identity[:, :])

            # occupancy row: log1p(counts).  comb rows are stored permuted as
            # [valence(16 rows), occ] (host permutes fW1 / ln_c to match) because
            # SBUF partition-offset writes must be 32-aligned -- the occ row lands
            # in partition 16 via an SBUF->SBUF DMA instead.
            occ_sb = work.tile([1, CTX], BF16, tag="work")
            nc.scalar.activation(out=occ_sb[:], in_=counts_ps[:1, :],
                                 func=AF.Ln, bias=1.0)
            nc.sync.dma_start(out=combT[H:H + 1, :], in_=occ_sb[:1, :])
            nc.scalar.activation(out=combT[0:H, :], in_=val_ps[:], func=AF.Tanh,
                                 bias=pb2_sb[:H, 0:1])

            # ---------- LN_c over the 17 rows ----------
            sum17 = ps.tile([1, CTX], F32, tag="ps")
            sumsq17 = ps.tile([1, CTX], F32, tag="ps")
            sq17 = work.tile([H + 1, CTX], BF16, tag="work")
            nc.scalar.activation(out=sq17[:], in_=combT[:], func=AF.Square)
            for n0, nsz in NSPLIT:
                nc.tensor.matmul(sum17[:1, n0:n0 + nsz], ones_col[:H + 1, :],
                                 combT[:, n0:n0 + nsz], start=True, stop=True)
                nc.tensor.matmul(sumsq17[:1, n0:n0 + nsz], ones_col[:H + 1, :],
                                 sq17[:, n0:n0 + nsz], start=True, stop=True)
            m17 = work.tile([1, CTX], BF16, tag="work")
            nc.scalar.activation(out=m17[:], in_=sum17[:1, :], func=AF.Copy,
                                 scale=1.0 / (H + 1))
            msq17 = work.tile([1, CTX], F32, tag="work")
            nc.scalar.activation(out=msq17[:], in_=sum17[:1, :], func=AF.Square,
                                 scale=1.0 / (H + 1))
            varp17 = work.tile([1, CTX], F32, tag="work")
            nc.vector.tensor_scalar_mul(varp17[:], sumsq17[:1, :], 1.0 / (H + 1))
            nc.vector.tensor_tensor(out=varp17[:], in0=varp17[:], in1=msq17[:],
                                    op=ALU.subtract)
            varp17_bf = work.tile([1, CTX], BF16, tag="work")
            nc.scalar.activation(out=varp17_bf[:], in_=varp17[:],
                                 func=AF.Abs_reciprocal_sqrt, bias=eps_sb[:1, :])
            mean_bc17 = ps.tile([H + 1, CTX], F32, tag="ps")
            rstd_bc17 = ps.tile([H + 1, CTX], F32, tag="ps")
            for n0, nsz in NSPLIT:
                nc.tensor.matmul(mean_bc17[:, n0:n0 + nsz], ones_row[:1, :H + 1],
                                 m17[:1, n0:n0 + nsz], start=True, stop=True)
                nc.tensor.matmul(rstd_bc17[:, n0:n0 + nsz], ones_row[:1, :H + 1],
                                 varp17_bf[:1, n0:n0 + nsz], start=True, stop=True)
            comb2f = work.tile([H + 1, CTX], F32, tag="work")
            nc.vector.tensor_tensor(out=comb2f[:], in0=combT[:], in1=mean_bc17[:],
                                    op=ALU.subtract)
            nc.vector.tensor_tensor(out=comb2f[:], in0=comb2f[:],
                                    in1=rstd_bc17[:], op=ALU.mult)
            comb2 = work.tile([H + 1, CTX], BF16, tag="work")
            nc.vector.tensor_scalar(comb2[:], comb2f[:], lncw_sb[:, 0:1],
                                    lncb_sb[:, 0:1], op0=ALU.mult, op1=ALU.add)

            # ---------- layers D, E, F ----------
            h2t = []
            for m, (mo, msz) in enumerate(FFN_CH):
                h2_ps = ps.tile([msz, CTX], F32, tag="ps")
                for n0, nsz in NSPLIT:
                    nc.tensor.matmul(h2_ps[:, n0:n0 + nsz],
                                     fW1_sb[:, mo:mo + msz],
                                     comb2[:, n0:n0 + nsz], start=True, stop=True)
                h2 = work.tile([msz, CTX], BF16, tag="work")
                nc.scalar.activation(out=h2[:], in_=h2_ps[:], func=AF.Gelu,
                                     bias=fb1_sb[:msz, m:m + 1])
                h2t.append(h2)
            h3t = []
            for m, (mo, msz) in enumerate(FFNH_CH):
                h3_ps = ps.tile([msz, CTX], F32, tag="ps")
                for n0, nsz in NSPLIT:
                    for k, (o, sz) in enumerate(FFN_CH):
                        nc.tensor.matmul(h3_ps[:, n0:n0 + nsz],
                                         fW2_sb[k][:, mo:mo + msz],
                                         h2t[k][:, n0:n0 + nsz],
                                         start=(k == 0),
                                         stop=(k == len(FFN_CH) - 1))
                h3 = work.tile([msz, CTX], BF16, tag="work")
                nc.scalar.activation(out=h3[:], in_=h3_ps[:], func=AF.Gelu,
                                     bias=fb2_sb[:msz, m:m + 1])
                h3t.append(h3)
            mod_ps = ps.tile([1, CTX], F32, tag="ps")
            for n0, nsz in NSPLIT:
                for k, (o, sz) in enumerate(FFNH_CH):
                    nc.tensor.matmul(mod_ps[:1, n0:n0 + nsz], fW3_sb[k][:, :],
                                     h3t[k][:, n0:n0 + nsz],
                                     start=(k == 0), stop=(k == len(FFNH_CH) - 1))
            mod_row = work.tile([1, CTX], F32, tag="work")
            nc.scalar.activation(out=mod_row[:], in_=mod_ps[:1, :], func=AF.Tanh,
                                 bias=fb3_sb[:1, 0:1])

            # mod row -> per-chunk columns (PE transpose of [1,128] slices)
            modc_ps = pcol.tile([128, CTX_CHUNKS], F32, tag="pc")
            for c in range(CTX_CHUNKS):
                nc.tensor.transpose(modc_ps[:, c:c + 1],
                                    mod_row[:1, ts(c, 128)], identity[:1, :1])
            nc.vector.tensor_copy(modv16[:], modc_ps[:])   # cast f32 -> f16

            # ---------- gather per-token mod via one-hot matmuls ----------
            # mod_tok row = modv.T @ OT  (contract over ctx).  Done as two
            # [1, 1024] psum rows; each 512-slice is one psum bank whose
            # 8-matmul accumulation group completes before the next group's
            # start=True clears the bank's has_written bits.
            mtok_sb = singles.tile([1, S], F32, tag="mtok", name=f"mtok{_mrep}")
            for half in range(2):
                row_ps = ps.tile([1, 1024], F32, tag="ps", name=f"grow{half}")
                for n0 in (0, 512):
                    for c in range(CTX_CHUNKS):
                        nc.tensor.matmul(
                            row_ps[:1, n0:n0 + 512],
                            modv16[:, c:c + 1],
                            ot_sb[c][:, half * 1024 + n0:half * 1024 + n0 + 512],
                            start=(c == 0), stop=(c == CTX_CHUNKS - 1))
                nc.vector.tensor_copy(
                    mtok_sb[:1, half * 1024:(half + 1) * 1024], row_ps[:1, :])
            gath_ps = pcol.tile([128, S_TILES], F32, tag="pc")
            for t in range(S_TILES):
                nc.tensor.transpose(gath_ps[:, t:t + 1],
                                    mtok_sb[:1, ts(t, 128)], identity[:1, :1])
            nc.scalar.activation(out=scales_sb[:], in_=gath_ps[:], func=AF.Copy,
                                 bias=1.0, scale=EPSILON)

        if not stream:
            N_CH = 0
        else:
            N_CH = N_CHUNKS
        # ---------- the memory-bound scale of attention_scores ----------
        for j in range(N_CH * stream_rep):
            j = j % N_CHUNKS
            r0 = j * ROWS_PER_CHUNK
            src = scores[r0:r0 + ROWS_PER_CHUNK, :].rearrange(
                "(p t) k -> p t k", p=128)
            dst = out[r0:r0 + ROWS_PER_CHUNK, :].rearrange(
                "(p t) k -> p t k", p=128)
            sc = sc_pool.tile([128, SUB_TILES, S], F16, tag="sc")
            nc.sync.dma_start(out=sc[:], in_=src)
            for t in range(SUB_TILES):
                qt = (j % (S // ROWS_PER_CHUNK)) * SUB_TILES + t
                if t % 4 == 3:
                    nc.scalar.activation(out=sc[:, t, :], in_=sc[:, t, :],
                                         func=AF.Copy,
                                         scale=scales_sb[:, qt:qt + 1])
                else:
                    nc.vector.tensor_scalar_mul(sc[:, t, :], sc[:, t, :],
                                                scales_sb[:, qt:qt + 1])
            nc.sync.dma_start(out=dst, in_=sc[:])

    nc.finalize()
    return nc


_NC = None


def _get_nc():
    global _NC
    if _NC is None:
        _NC = build_nc()
    return _NC


def _cols(v, ncols):
    out = np.zeros((128, ncols), np.float32)
    v = v.reshape(-1)
    for k, (o, sz) in enumerate(_chunks(len(v))):
        out[:sz, k] = v[o:o + sz]
    return out


def build_in_maps(inputs):
    ids = np.asarray(inputs["input_ids"]).astype(np.int64)

    iota_cols = np.ascontiguousarray(
        np.arange(CTX, dtype=np.float32).reshape(CTX_CHUNKS, 128).T)
    # gather slot t' = c*128 + p must hold token q = row (p-major chunks):
    # chunk j rows are r0 + p*SUB_TILES + t, q = (c//SUB_TILES)*ROWS_PER_CHUNK
    # + p*SUB_TILES + (c%SUB_TILES) with c = scales column.
    cc, pp = np.meshgrid(np.arange(S_TILES), np.arange(128), indexing="ij")
    qmap = ((cc // SUB_TILES) * ROWS_PER_CHUNK + pp * SUB_TILES
            + (cc % SUB_TILES)).reshape(-1)

    f32 = lambda x: np.ascontiguousarray(np.asarray(x, dtype=np.float32))
    bf16 = lambda x: np.ascontiguousarray(
        np.asarray(x, dtype=np.float32).astype(ml_dtypes.bfloat16))
    embT = bf16(np.asarray(inputs["emb_W"]).T)
    lnvw = np.zeros((128, 3), np.float32)
    lnvb = np.zeros((128, 3), np.float32)
    wv = f32(inputs["ln_v_w"]).reshape(-1)
    bv = f32(inputs["ln_v_b"]).reshape(-1)
    for k, (o, sz) in enumerate(_chunks(VD)):
        lnvw[:sz, k] = wv[o:o + sz]
        lnvb[:sz, k] = bv[o:o + sz]

    common = {
        "iota_cols": iota_cols,
        "embT": embT,
        "pW1": bf16(inputs["pW1"]), "pb1": _cols(f32(inputs["pb1"]), 5),
        "pW2": bf16(inputs["pW2"]), "pb2": f32(inputs["pb2"]).reshape(-1, 1),
        "fW1": np.ascontiguousarray(np.roll(bf16(inputs["fW1"]), -1, axis=0)),
        "fb1": _cols(f32(inputs["fb1"]), 4),
        "fW2": bf16(inputs["fW2"]), "fb2": _cols(f32(inputs["fb2"]), 2),
        "fW3": bf16(inputs["fW3"]), "fb3": f32(inputs["fb3"]).reshape(1, -1),
        "lnvw": lnvw, "lnvb": lnvb,
        "lncw": np.roll(f32(inputs["ln_c_w"]), -1).reshape(-1, 1),
        "lncb": np.roll(f32(inputs["ln_c_b"]), -1).reshape(-1, 1),
    }

    scores = np.asarray(inputs["attention_scores"])
    scores_flat = scores.reshape(B * H, S, S)
    in_maps = []
    for i in range(N_CORES):
        b = i // (N_CORES // B)
        shard = np.ascontiguousarray(
            scores_flat[i * HEADS_PER_CORE:(i + 1) * HEADS_PER_CORE]
        ).reshape(SHARD_ROWS, S).astype(np.float16)
        m = dict(common)
        m["scores"] = shard
        m["ids_loc"] = ids[b][qmap].astype(np.float16).reshape(1, S)
        m["ids_rem"] = ids[1 - b].astype(np.float16).reshape(1, S)
        in_maps.append(m)
    return in_maps


def _run(inputs, **spmd_kwargs):
    in_maps = build_in_maps(inputs)
    nc = _get_nc()
    res = run_bass_kernel_spmd(nc, in_maps, core_ids=list(range(N_CORES)),
                               **spmd_kwargs)
    shards = [res.results[i]["out"] for i in range(N_CORES)]
    out = np.concatenate(shards, axis=0).reshape(B, H, S, S).astype(np.float32)
    return out, res


def kernel(**inputs) -> np.ndarray:
    return _run(inputs)[0]


if __name__ == "__main__":
    rng = np.random.default_rng(0)
    inputs = {
        "attention_scores": rng.standard_normal((B, H, S, S), dtype=np.float32),
        "input_ids": rng.integers(0, CTX, size=(B, S)),
        "emb_W": rng.standard_normal((CTX, VD), dtype=np.float32) * 0.05,
        "ln_v_w": np.ones(VD, np.float32), "ln_v_b": np.zeros(VD, np.float32),
        "pW1": rng.standard_normal((VD, 2 * VD), dtype=np.float32) * 0.05,
        "pb1": rng.standard_normal(2 * VD, dtype=np.float32) * 0.05,
        "pW2": rng.standard_normal((576, H), dtype=np.float32) * 0.04,
        "pb2": rng.standard_normal(H, dtype=np.float32) * 0.04,
        "ln_c_w": np.ones(H + 1, np.float32), "ln_c_b": np.zeros(H + 1, np.float32),
        "fW1": rng.standard_normal((H + 1, FFN), dtype=np.float32) * 0.2,
        "fb1": rng.standard_normal(FFN, dtype=np.float32) * 0.2,
        "fW2": rng.standard_normal((FFN, FFN // 2), dtype=np.float32) * 0.04,
        "fb2": rng.standard_normal(FFN // 2, dtype=np.float32) * 0.04,
        "fW3": rng.standard_normal((FFN // 2, 1), dtype=np.float32) * 0.06,
        "fb3": rng.standard_normal(1, dtype=np.float32) * 0.06,
    }
    out = kernel(**inputs)
    print("kernel output", out.shape, out.dtype, float(np.abs(out).mean()))


# revision 18
# speedup vs baseline: 1.7775x; 1.7775x over previous
"""Trainium2 Bass kernel for nn_MemoryWeightedAttention.

out[b,h,q,k] = attention_scores[b,h,q,k] * (1 + 0.066 * mod[b,q])

where mod[b,q] is a small LN/MLP pipeline applied to
(log1p(global_count[id]), tanh-MLP(LN(emb[id]))) -- i.e. mod depends ONLY on
the token id at (b,q) plus a global histogram of input_ids.  So we compute the
full pipeline once over the 1024-entry vocabulary table (in transposed layout:
features on partitions, vocab on the free dim -> no transposes between
layers), then gather per-token scales with one-hot matmuls, and stream the
attention_scores tensor (as fp16) through a per-partition scalar multiply.

Sharding: 8 cores, core i handles batch i//4, heads 4*(i%4) .. 4*(i%4)+4
(a contiguous [8192, 2048] row-slice of the flattened [65536, 2048] scores).
The tiny table pipeline is replicated on every core (no collectives).

Precision: the scores stream is fp16 (input quantization ~5e-4 rel) and the
table pipeline runs in bf16 on the PE (4x faster than fp32); the resulting
scale factor (1 + 0.066*tanh(...)) error is ~1e-4.  Total mean rel err
~5e-4, far under the 2e-2 gate.
"""

import sys

for _p in ("/opt/trn_rl_repo",):
    if _p not in sys.path:
        sys.path.insert(0, _p)

from contextlib import ExitStack

import numpy as np
import ml_dtypes

import concourse.bacc as bacc
import concourse.tile as tile
from concourse import mybir
from concourse.bass import ts
from concourse.bass_utils import run_bass_kernel_spmd
from concourse.masks import make_identity

F32 = mybir.dt.float32
F16 = mybir.dt.float16
F8 = mybir.dt.float8e4
BF16 = mybir.dt.bfloat16
AF = mybir.ActivationFunctionType
ALU = mybir.AluOpType

B, H, S = 2, 16, 2048
CTX = 1024
VD = 288          # valence dim
FFN = 512
EPSILON = 0.066
LN_EPS = 1e-5

N_CORES = 8
HEADS_PER_CORE = H * B // N_CORES          # 4
SHARD_ROWS = HEADS_PER_CORE * S            # 8192
N_TOK = B * S                              # 4096 (global, for counts)
S_TILES = S // 128                         # 16 token tiles per core's batch
TOK_TILES = N_TOK // 128                   # 32 global token tiles
CTX_CHUNKS = CTX // 128                    # 8
ROWS_PER_CHUNK = 512                       # score rows per DMA chunk
N_CHUNKS = SHARD_ROWS // ROWS_PER_CHUNK    # 16
SUB_TILES = ROWS_PER_CHUNK // 128          # 4
STREAM_BUFS = 8


def _chunks(n, c=128):
    out = []
    o = 0
    while o < n:
        out.append((o, min(c, n - o)))
        o += c
    return out


def build_nc(mod=True, stream=True, mod_rep=1, stream_rep=1):
    nc = bacc.Bacc("TRN2", target_bir_lowering=False, debug=False,
                   num_devices=N_CORES)

    dt = nc.dram_tensor
    scores = dt("scores", [SHARD_ROWS, S], F16, kind="ExternalInput")
    ids_loc = dt("ids_loc", [1, S], F16, kind="ExternalInput")
    ids_rem = dt("ids_rem", [1, S], F16, kind="ExternalInput")
    iota_cols = dt("iota_cols", [128, CTX_CHUNKS], F32, kind="ExternalInput")
    embT = dt("embT", [VD, CTX], BF16, kind="ExternalInput")
    pW1 = dt("pW1", [VD, 2 * VD], BF16, kind="ExternalInput")
    pb1 = dt("pb1", [128, 5], F32, kind="ExternalInput")
    pW2 = dt("pW2", [2 * VD, H], BF16, kind="ExternalInput")
    pb2 = dt("pb2", [H, 1], F32, kind="ExternalInput")
    fW1 = dt("fW1", [H + 1, FFN], BF16, kind="ExternalInput")
    fb1 = dt("fb1", [128, 4], F32, kind="ExternalInput")
    fW2 = dt("fW2", [FFN, FFN // 2], BF16, kind="ExternalInput")
    fb2 = dt("fb2", [128, 2], F32, kind="ExternalInput")
    fW3 = dt("fW3", [FFN // 2, 1], BF16, kind="ExternalInput")
    fb3 = dt("fb3", [1, 1], F32, kind="ExternalInput")
    lnvw = dt("lnvw", [128, 3], F32, kind="ExternalInput")   # VD cols chunked
    lnvb = dt("lnvb", [128, 3], F32, kind="ExternalInput")
    lncw = dt("lncw", [H + 1, 1], F32, kind="ExternalInput")
    lncb = dt("lncb", [H + 1, 1], F32, kind="ExternalInput")
    out = dt("out", [SHARD_ROWS, S], F16, kind="ExternalOutput")

    VD_CH = _chunks(VD)            # [(0,128),(128,128),(256,32)]
    VD2_CH = _chunks(2 * VD)       # 576 -> 5 chunks
    FFN_CH = _chunks(FFN)          # 512 -> 4
    FFNH_CH = _chunks(FFN // 2)    # 256 -> 2
    NSPLIT = [(0, 512), (512, 512)]   # vocab free-dim split

    with tile.TileContext(nc) as tc, ExitStack() as ctx:
        singles = ctx.enter_context(tc.tile_pool(name="singles", bufs=1))
        work = ctx.enter_context(tc.tile_pool(name="work", bufs=8))
        otpool = ctx.enter_context(tc.tile_pool(name="otpool", bufs=1))
        ps = ctx.enter_context(tc.tile_pool(name="ps", bufs=3, space="PSUM"))
        pcol = ctx.enter_context(tc.tile_pool(name="pcol", bufs=1, space="PSUM"))
        sc_pool = ctx.enter_context(tc.tile_pool(name="sc", bufs=STREAM_BUFS))

        # ---------- constants / weights into SBUF ----------
        _uid = [0]

        def load(shape, src, dtype=F32):
            _uid[0] += 1
            t = singles.tile(shape, dtype, tag=f"s{_uid[0]}", name=f"s{_uid[0]}")
            nc.sync.dma_start(out=t[:], in_=src)
            return t

        def stile(shape, dtype=F32):
            _uid[0] += 1
            return singles.tile(shape, dtype, tag=f"s{_uid[0]}", name=f"s{_uid[0]}")

        embT_sb = []
        for o, sz in VD_CH:
            _t = work.tile([sz, CTX], BF16, tag="work", name=f"embT{o}")
            nc.sync.dma_start(out=_t[:], in_=embT[o:o + sz, :])
            embT_sb.append(_t)
        lnvw_sb = load([128, 3], lnvw[:, :])
        lnvb_sb = load([128, 3], lnvb[:, :])
        iota_cols_sb = load([128, CTX_CHUNKS], iota_cols[:, :], F32)
        ids_loc_b = stile([128, S], F16)
        nc.sync.dma_start(out=ids_loc_b[:], in_=ids_loc.ap().to_broadcast((128, S)))
        ids_rem_b = stile([128, S], F16)
        nc.sync.dma_start(out=ids_rem_b[:], in_=ids_rem.ap().to_broadcast((128, S)))
        pW1_sb = [load([sz, 2 * VD], pW1[o:o + sz, :], BF16) for o, sz in VD_CH]
        pW2_sb = [load([sz, H], pW2[o:o + sz, :], BF16) for o, sz in VD2_CH]
        fW1_sb = load([H + 1, FFN], fW1[:, :], BF16)
        fW2_sb = [load([sz, FFN // 2], fW2[o:o + sz, :], BF16) for o, sz in FFN_CH]
        fW3_sb = [load([sz, 1], fW3[o:o + sz, :], BF16) for o, sz in FFNH_CH]
        pb1_sb = load([128, 5], pb1[:, :])
        pb2_sb = load([H, 1], pb2[:, :])
        fb1_sb = load([128, 4], fb1[:, :])
        fb2_sb = load([128, 2], fb2[:, :])
        fb3_sb = load([1, 1], fb3[:, :])
        lncw_sb = load([H + 1, 1], lncw[:, :])
        lncb_sb = load([H + 1, 1], lncb[:, :])

        ones_col = stile([128, 1], BF16)
        nc.vector.memset(ones_col[:], 1.0)
        ones_row = stile([1, 128], BF16)
        nc.vector.memset(ones_row[:], 1.0)
        identity = stile([128, 128], F32)
        make_identity(nc, identity[:])

        eps_sb = stile([128, 1], F32)
        nc.vector.memset(eps_sb[:], LN_EPS)
        modv16 = stile([128, CTX_CHUNKS], F16)
        scales_sb = stile([128, S_TILES], F32)

        if not mod:
            nc.vector.memset(scales_sb[:], 1.0)
        for _mrep in range(mod_rep if mod else 0):
            # ---------- LN_v of emb table, transposed layout ----------
            # sum / sumsq rows over the 288 feature partitions via ones-lhsT matmul
            sum_ps = ps.tile([1, CTX], F32, tag="ps")
            sumsq_ps = ps.tile([1, CTX], F32, tag="ps")
            sq_t = []
            for k, (o, sz) in enumerate(VD_CH):
                sq = work.tile([sz, CTX], BF16, tag="work")
                nc.scalar.activation(out=sq[:], in_=embT_sb[k][:], func=AF.Square)
                sq_t.append(sq)
            for n0, nsz in NSPLIT:
                for k, (o, sz) in enumerate(VD_CH):
                    nc.tensor.matmul(sum_ps[:1, n0:n0 + nsz], ones_col[:sz, :],
                                     embT_sb[k][:, n0:n0 + nsz],
                                     start=(k == 0), stop=(k == len(VD_CH) - 1))
                for k, (o, sz) in enumerate(VD_CH):
                    nc.tensor.matmul(sumsq_ps[:1, n0:n0 + nsz], ones_col[:sz, :],
                                     sq_t[k][:, n0:n0 + nsz],
                                     start=(k == 0), stop=(k == len(VD_CH) - 1))


            # ---------- LN_v stats rows ----------
            m_row = work.tile([1, CTX], BF16, tag="work")
            nc.scalar.activation(out=m_row[:], in_=sum_ps[:1, :], func=AF.Copy,
                                 scale=1.0 / VD)
            msq_row = work.tile([1, CTX], F32, tag="work")
            nc.scalar.activation(out=msq_row[:], in_=sum_ps[:1, :], func=AF.Square,
                                 scale=1.0 / VD)
            varp = work.tile([1, CTX], F32, tag="work")
            nc.vector.tensor_scalar_mul(varp[:], sumsq_ps[:1, :], 1.0 / VD)
            nc.vector.tensor_tensor(out=varp[:], in0=varp[:], in1=msq_row[:],
                                    op=ALU.subtract)
            varp_bf = work.tile([1, CTX], BF16, tag="work")
            nc.scalar.activation(out=varp_bf[:], in_=varp[:],
                                 func=AF.Abs_reciprocal_sqrt, bias=eps_sb[:1, :])

            # broadcast mean / rstd rows across 128 partitions (bf16 PE matmul)
            mean_bc = ps.tile([128, CTX], F32, tag="ps")
            rstd_bc = ps.tile([128, CTX], F32, tag="ps")
            for n0, nsz in NSPLIT:
                nc.tensor.matmul(mean_bc[:, n0:n0 + nsz], ones_row[:1, :],
                                 m_row[:1, n0:n0 + nsz], start=True, stop=True)
                nc.tensor.matmul(rstd_bc[:, n0:n0 + nsz], ones_row[:1, :],
                                 varp_bf[:1, n0:n0 + nsz], start=True, stop=True)

            # ---------- normalized emb table E'T ----------
            e1t = []
            for k, (o, sz) in enumerate(VD_CH):
                e1 = work.tile([sz, CTX], F32, tag="work")
                nc.vector.tensor_tensor(out=e1[:], in0=embT_sb[k][:],
                                        in1=mean_bc[:sz, :], op=ALU.subtract)
                nc.vector.tensor_tensor(out=e1[:], in0=e1[:], in1=rstd_bc[:sz, :],
                                        op=ALU.mult)
                e1b = work.tile([sz, CTX], BF16, tag="work")
                nc.vector.tensor_scalar(e1b[:], e1[:], lnvw_sb[:sz, k:k + 1],
                                        lnvb_sb[:sz, k:k + 1],
                                        op0=ALU.mult, op1=ALU.add)
                e1t.append(e1b)

            # ---------- global token histogram (counts) ----------
            # One-hot tiles OT[c][p, t] = (ids[t] == c*128+p) serve double
            # duty: their fused accum_out column is this batch's histogram
            # contribution (sum over tokens), and the tiles feed the mod
            # gather matmuls at the end of the pipeline.  The remote batch
            # contributes via accum-only one-hots into a reused scratch tile.
            acc_loc = stile([128, CTX_CHUNKS], F32)
            acc_rem = stile([128, CTX_CHUNKS], F32)
            ot_sb = []
            for c in range(CTX_CHUNKS):
                _o = otpool.tile([128, S], F8, tag=f"ot{c}", name=f"ot{c}")
                nc.vector.tensor_scalar(_o[:], ids_loc_b[:],
                                        iota_cols_sb[:, c:c + 1], None,
                                        op0=ALU.is_equal)
                ot_sb.append(_o)
            for c in range(CTX_CHUNKS):
                otr = otpool.tile([128, S], F8, tag="otr", name=f"otr{c}")
                nc.vector.tensor_scalar(otr[:], ids_rem_b[:],
                                        iota_cols_sb[:, c:c + 1], 0.0,
                                        op0=ALU.is_equal, op1=ALU.add,
                                        accum_out=acc_rem[:, c:c + 1])
            counts_cs = stile([128, CTX_CHUNKS], F32)
            nc.vector.tensor_tensor(out=counts_cs[:], in0=acc_loc[:],
                                    in1=acc_rem[:], op=ALU.add)

            # ---------- layer A (gelu(E' @ pW1 + pb1)) fused with ----------
            # ---------- layer B (tanh(H1 @ pW2 + pb2)) accumulation ----------
            combT = work.tile([H + 1, CTX], BF16, tag="work")
            val_ps = ps.tile([H, CTX], F32, tag="ps")
            for m, (mo, msz) in enumerate(VD2_CH):
                h1_ps = ps.tile([msz, CTX], F32, tag="ps")
                for n0, nsz in NSPLIT:
                    for k, (o, sz) in enumerate(VD_CH):
                        nc.tensor.matmul(h1_ps[:, n0:n0 + nsz],
                                         pW1_sb[k][:, mo:mo + msz],
                                         e1t[k][:, n0:n0 + nsz],
                                         start=(k == 0),
                                         stop=(k == len(VD_CH) - 1))
                h1 = work.tile([msz, CTX], BF16, tag="work")
                nc.scalar.activation(out=h1[:], in_=h1_ps[:], func=AF.Gelu,
                                     bias=pb1_sb[:msz, m:m + 1])
                if m < CTX_CHUNKS:
                    nc.scalar.activation(out=ot_sb[m][:], in_=ot_sb[m][:],
                                         func=AF.Copy,
                                         accum_out=acc_loc[:, m:m + 1])
                for n0, nsz in NSPLIT:
                    nc.tensor.matmul(val_ps[:, n0:n0 + nsz], pW2_sb[m][:, :],
                                     h1[:, n0:n0 + nsz],
                                     start=(m == 0), stop=(m == len(VD2_CH) - 1))

            for c in range(len(VD2_CH), CTX_CHUNKS):
                nc.scalar.activation(out=ot_sb[c][:], in_=ot_sb[c][:],
                                     func=AF.Copy,
                                     accum_out=acc_loc[:, c:c + 1])
            counts_ps = ps.tile([1, CTX], F32, tag="ps")
            for c in range(CTX_CHUNKS):
                nc.tensor.transpose(counts_ps[:1, ts(c, 128)],
                                    counts_cs[:, c:c + 1], identity[:, :])

            # occupancy row: log1p(counts).  comb rows are stored permuted as
            # [valence(16 rows), occ] (host permutes fW1 / ln_c to match) because
            # SBUF partition-offset writes must be 32-aligned -- the occ row lands
            # in partition 16 via an SBUF->SBUF DMA instead.
            occ_sb = work.tile([1, CTX], BF16, tag="work")
            nc.scalar.activation(out=occ_sb[:], in_=counts_ps[:1, :],
                                 func=AF.Ln, bias=1.0)
            nc.sync.dma_start(out=combT[H:H + 1, :], in_=occ_sb[:1, :])
            nc.scalar.activation(out=combT[0:H, :], in_=val_ps[:], func=AF.Tanh,
                                 bias=pb2_sb[:H, 0:1])

            # ---------- LN_c over the 17 rows ----------
            sum17 = ps.tile([1, CTX], F32, tag="ps")
            sumsq17 = ps.tile([1, CTX], F32, tag="ps")
            sq17 = work.tile([H + 1, CTX], BF16, tag="work")
            nc.scalar.activation(out=sq17[:], in_=combT[:], func=AF.Square)
            for n0, nsz in NSPLIT:
                nc.tensor.matmul(sum17[:1, n0:n0 + nsz], ones_col[:H + 1, :],
                                 combT[:, n0:n0 + nsz], start=True, stop=True)
                nc.tensor.matmul(sumsq17[:1, n0:n0 + nsz], ones_col[:H + 1, :],
                                 sq17[:, n0:n0 + nsz], start=True, stop=True)
            m17 = work.tile([1, CTX], BF16, tag="work")
            nc.scalar.activation(out=m17[:], in_=sum17[:1, :], func=AF.Copy,
                                 scale=1.0 / (H + 1))
            msq17 = work.tile([1, CTX], F32, tag="work")
            nc.scalar.activation(out=msq17[:], in_=sum17[:1, :], func=AF.Square,
                                 scale=1.0 / (H + 1))
            varp17 = work.tile([1, CTX], F32, tag="work")
            nc.vector.tensor_scalar_mul(varp17[:], sumsq17[:1, :], 1.0 / (H + 1))
            nc.vector.tensor_tensor(out=varp17[:], in0=varp17[:], in1=msq17[:],
                                    op=ALU.subtract)
            varp17_bf = work.tile([1, CTX], BF16, tag="work")
            nc.scalar.activation(out=varp17_bf[:], in_=varp17[:],
                                 func=AF.Abs_reciprocal_sqrt, bias=eps_sb[:1, :])
            mean_bc17 = ps.tile([H + 1, CTX], F32, tag="ps")
            rstd_bc17 = ps.tile([H + 1, CTX], F32, tag="ps")
            for n0, nsz in NSPLIT:
                nc.tensor.matmul(mean_bc17[:, n0:n0 + nsz], ones_row[:1, :H + 1],
                                 m17[:1, n0:n0 + nsz], start=True, stop=True)
                nc.tensor.matmul(rstd_bc17[:, n0:n0 + nsz], ones_row[:1, :H + 1],
                                 varp17_bf[:1, n0:n0 + nsz], start=True, stop=True)
            comb2f = work.tile([H + 1, CTX], F32, tag="work")
            nc.vector.tensor_tensor(out=comb2f[:], in0=combT[:], in1=mean_bc17[:],
                                    op=ALU.subtract)
            nc.vector.tensor_tensor(out=comb2f[:], in0=comb2f[:],
                                    in1=rstd_bc17[:], op=ALU.mult)
            comb2 = work.tile([H + 1, CTX], BF16, tag="work")
            nc.vector.tensor_scalar(comb2[:], comb2f[:], lncw_sb[:, 0:1],
                                    lncb_sb[:, 0:1], op0=ALU.mult, op1=ALU.add)

            # ---------- layers D, E, F ----------
            h2t = []
            for m, (mo, msz) in enumerate(FFN_CH):
                h2_ps = ps.tile([msz, CTX], F32, tag="ps")
                for n0, nsz in NSPLIT:
                    nc.tensor.matmul(h2_ps[:, n0:n0 + nsz],
                                     fW1_sb[:, mo:mo + msz],
                                     comb2[:, n0:n0 + nsz], start=True, stop=True)
                h2 = work.tile([msz, CTX], BF16, tag="work")
                nc.scalar.activation(out=h2[:], in_=h2_ps[:], func=AF.Gelu,
                                     bias=fb1_sb[:msz, m:m + 1])
                h2t.append(h2)
            h3t = []
            for m, (mo, msz) in enumerate(FFNH_CH):
                h3_ps = ps.tile([msz, CTX], F32, tag="ps")
                for n0, nsz in NSPLIT:
                    for k, (o, sz) in enumerate(FFN_CH):
                        nc.tensor.matmul(h3_ps[:, n0:n0 + nsz],
                                         fW2_sb[k][:, mo:mo + msz],
                                         h2t[k][:, n0:n0 + nsz],
                                         start=(k == 0),
                                         stop=(k == len(FFN_CH) - 1))
                h3 = work.tile([msz, CTX], BF16, tag="work")
                nc.scalar.activation(out=h3[:], in_=h3_ps[:], func=AF.Gelu,
                                     bias=fb2_sb[:msz, m:m + 1])
                h3t.append(h3)
            mod_ps = ps.tile([1, CTX], F32, tag="ps")
            for n0, nsz in NSPLIT:
                for k, (o, sz) in enumerate(FFNH_CH):
                    nc.tensor.matmul(mod_ps[:1, n0:n0 + nsz], fW3_sb[k][:, :],
                                     h3t[k][:, n0:n0 + nsz],
                                     start=(k == 0), stop=(k == len(FFNH_CH) - 1))
            mod_row = work.tile([1, CTX], F32, tag="work")
            nc.scalar.activation(out=mod_row[:], in_=mod_ps[:1, :], func=AF.Tanh,
                                 bias=fb3_sb[:1, 0:1])

            # mod row -> per-chunk columns (PE transpose of [1,128] slices)
            modc_ps = pcol.tile([128, CTX_CHUNKS], F32, tag="pc")
            for c in range(CTX_CHUNKS):
                nc.tensor.transpose(modc_ps[:, c:c + 1],
                                    mod_row[:1, ts(c, 128)], identity[:1, :1])
            nc.vector.tensor_copy(modv16[:], modc_ps[:])   # cast f32 -> f16

            # ---------- gather per-token mod via one-hot matmuls ----------
            # mod_tok row = modv.T @ OT  (contract over ctx).  Done as two
            # [1, 1024] psum rows; each 512-slice is one psum bank whose
            # 8-matmul accumulation group completes before the next group's
            # start=True clears the bank's has_written bits.
            mtok_sb = singles.tile([1, S], F32, tag="mtok", name=f"mtok{_mrep}")
            for half in range(2):
                row_ps = ps.tile([1, 1024], F32, tag="ps", name=f"grow{half}")
                for n0 in (0, 512):
                    for c in range(CTX_CHUNKS):
                        nc.tensor.matmul(
                            row_ps[:1, n0:n0 + 512],
                            modv16[:, c:c + 1],
                            ot_sb[c][:, half * 1024 + n0:half * 1024 + n0 + 512],
                            start=(c == 0), stop=(c == CTX_CHUNKS - 1))
                nc.vector.tensor_copy(
                    mtok_sb[:1, half * 1024:(half + 1) * 1024], row_ps[:1, :])
            gath_ps = pcol.tile([128, S_TILES], F32, tag="pc")
            for t in range(S_TILES):
                nc.tensor.transpose(gath_ps[:, t:t + 1],
                                    mtok_sb[:1, ts(t, 128)], identity[:1, :1])
            nc.scalar.activation(out=scales_sb[:], in_=gath_ps[:], func=AF.Copy,
                                 bias=1.0, scale=EPSILON)

        if not stream:
            N_CH = 0
        else:
            N_CH = N_CHUNKS
        # ---------- the memory-bound scale of attention_scores ----------
        for j in range(N_CH * stream_rep):
            j = j % N_CHUNKS
            r0 = j * ROWS_PER_CHUNK
            src = scores[r0:r0 + ROWS_PER_CHUNK, :].rearrange(
                "(p t) k -> p t k", p=128)
            dst = out[r0:r0 + ROWS_PER_CHUNK, :].rearrange(
                "(p t) k -> p t k", p=128)
            sc = sc_pool.tile([128, SUB_TILES, S], F16, tag="sc")
            nc.sync.dma_start(out=sc[:], in_=src)
            for t in range(SUB_TILES):
                qt = (j % (S // ROWS_PER_CHUNK)) * SUB_TILES + t
                if t % 4 == 3:
                    nc.scalar.activation(out=sc[:, t, :], in_=sc[:, t, :],
                                         func=AF.Copy,
                                         scale=scales_sb[:, qt:qt + 1])
                else:
                    nc.vector.tensor_scalar_mul(sc[:, t, :], sc[:, t, :],
                                                scales_sb[:, qt:qt + 1])
            nc.sync.dma_start(out=dst, in_=sc[:])

    nc.finalize()
    return nc


_NC = None


def _get_nc():
    global _NC
    if _NC is None:
        _NC = build_nc()
    return _NC


def _cols(v, ncols):
    out = np.zeros((128, ncols), np.float32)
    v = v.reshape(-1)
    for k, (o, sz) in enumerate(_chunks(len(v))):
        out[:sz, k] = v[o:o + sz]
    return out


def build_in_maps(inputs):
    ids = np.asarray(inputs["input_ids"]).astype(np.int64)

    iota_cols = np.ascontiguousarray(
        np.arange(CTX, dtype=np.float32).reshape(CTX_CHUNKS, 128).T)
    # gather slot t' = c*128 + p must hold token q = row (p-major chunks):
    # chunk j rows are r0 + p*SUB_TILES + t, q = (c//SUB_TILES)*ROWS_PER_CHUNK
    # + p*SUB_TILES + (c%SUB_TILES) with c = scales column.
    cc, pp = np.meshgrid(np.arange(S_TILES), np.arange(128), indexing="ij")
    qmap = ((cc // SUB_TILES) * ROWS_PER_CHUNK + pp * SUB_TILES
            + (cc % SUB_TILES)).reshape(-1)

    f32 = lambda x: np.ascontiguousarray(np.asarray(x, dtype=np.float32))
    bf16 = lambda x: np.ascontiguousarray(
        np.asarray(x, dtype=np.float32).astype(ml_dtypes.bfloat16))
    embT = bf16(np.asarray(inputs["emb_W"]).T)
    lnvw = np.zeros((128, 3), np.float32)
    lnvb = np.zeros((128, 3), np.float32)
    wv = f32(inputs["ln_v_w"]).reshape(-1)
    bv = f32(inputs["ln_v_b"]).reshape(-1)
    for k, (o, sz) in enumerate(_chunks(VD)):
        lnvw[:sz, k] = wv[o:o + sz]
        lnvb[:sz, k] = bv[o:o + sz]

    common = {
        "iota_cols": iota_cols,
        "embT": embT,
        "pW1": bf16(inputs["pW1"]), "pb1": _cols(f32(inputs["pb1"]), 5),
        "pW2": bf16(inputs["pW2"]), "pb2": f32(inputs["pb2"]).reshape(-1, 1),
        "fW1": np.ascontiguousarray(np.roll(bf16(inputs["fW1"]), -1, axis=0)),
        "fb1": _cols(f32(inputs["fb1"]), 4),
        "fW2": bf16(inputs["fW2"]), "fb2": _cols(f32(inputs["fb2"]), 2),
        "fW3": bf16(inputs["fW3"]), "fb3": f32(inputs["fb3"]).reshape(1, -1),
        "lnvw": lnvw, "lnvb": lnvb,
        "lncw": np.roll(f32(inputs["ln_c_w"]), -1).reshape(-1, 1),
        "lncb": np.roll(f32(inputs["ln_c_b"]), -1).reshape(-1, 1),
    }

    scores = np.asarray(inputs["attention_scores"])
    scores_flat = scores.reshape(B * H, S, S)
    in_maps = []
    for i in range(N_CORES):
        b = i // (N_CORES // B)
        shard = np.ascontiguousarray(
            scores_flat[i * HEADS_PER_CORE:(i + 1) * HEADS_PER_CORE]
        ).reshape(SHARD_ROWS, S).astype(np.float16)
        m = dict(common)
        m["scores"] = shard
        m["ids_loc"] = ids[b][qmap].astype(np.float16).reshape(1, S)
        m["ids_rem"] = ids[1 - b].astype(np.float16).reshape(1, S)
        in_maps.append(m)
    return in_maps


def _run(inputs, **spmd_kwargs):
    in_maps = build_in_maps(inputs)
    nc = _get_nc()
    res = run_bass_kernel_spmd(nc, in_maps, core_ids=list(range(N_CORES)),
                               **spmd_kwargs)
    shards = [res.results[i]["out"] for i in range(N_CORES)]
    out = np.concatenate(shards, axis=0).reshape(B, H, S, S).astype(np.float32)
    return out, res


def kernel(**inputs) -> np.ndarray:
    return _run(inputs)[0]


if __name__ == "__main__":
    rng = np.random.default_rng(0)
    inputs = {
        "attention_scores": rng.standard_normal((B, H, S, S), dtype=np.float32),
        "input_ids": rng.integers(0, CTX, size=(B, S)),
        "emb_W": rng.standard_normal((CTX, VD), dtype=np.float32) * 0.05,
        "ln_v_w": np.ones(VD, np.float32), "ln_v_b": np.zeros(VD, np.float32),
        "pW1": rng.standard_normal((VD, 2 * VD), dtype=np.float32) * 0.05,
        "pb1": rng.standard_normal(2 * VD, dtype=np.float32) * 0.05,
        "pW2": rng.standard_normal((576, H), dtype=np.float32) * 0.04,
        "pb2": rng.standard_normal(H, dtype=np.float32) * 0.04,
        "ln_c_w": np.ones(H + 1, np.float32), "ln_c_b": np.zeros(H + 1, np.float32),
        "fW1": rng.standard_normal((H + 1, FFN), dtype=np.float32) * 0.2,
        "fb1": rng.standard_normal(FFN, dtype=np.float32) * 0.2,
        "fW2": rng.standard_normal((FFN, FFN // 2), dtype=np.float32) * 0.04,
        "fb2": rng.standard_normal(FFN // 2, dtype=np.float32) * 0.04,
        "fW3": rng.standard_normal((FFN // 2, 1), dtype=np.float32) * 0.06,
        "fb3": rng.standard_normal(1, dtype=np.float32) * 0.06,
    }
    out = kernel(**inputs)
    print("kernel output", out.shape, out.dtype, float(np.abs(out).mean()))


# revision 19
# speedup vs baseline: 1.8996x; 1.0687x over previous
"""Trainium2 Bass kernel for nn_MemoryWeightedAttention.

out[b,h,q,k] = attention_scores[b,h,q,k] * (1 + 0.066 * mod[b,q])

where mod[b,q] is a small LN/MLP pipeline applied to
(log1p(global_count[id]), tanh-MLP(LN(emb[id]))) -- i.e. mod depends ONLY on
the token id at (b,q) plus a global histogram of input_ids.  So we compute the
full pipeline once over the 1024-entry vocabulary table (in transposed layout:
features on partitions, vocab on the free dim -> no transposes between
layers), then gather per-token scales with one-hot matmuls, and stream the
attention_scores tensor (as fp16) through a per-partition scalar multiply.

Sharding: 8 cores, core i handles batch i//4, heads 4*(i%4) .. 4*(i%4)+4
(a contiguous [8192, 2048] row-slice of the flattened [65536, 2048] scores).
The tiny table pipeline is replicated on every core (no collectives).

Precision: the scores stream is fp16 (input quantization ~5e-4 rel) and the
table pipeline runs in bf16 on the PE (4x faster than fp32); the resulting
scale factor (1 + 0.066*tanh(...)) error is ~1e-4.  Total mean rel err
~5e-4, far under the 2e-2 gate.
"""

import sys

for _p in ("/opt/trn_rl_repo",):
    if _p not in sys.path:
        sys.path.insert(0, _p)

from contextlib import ExitStack

import numpy as np
import ml_dtypes

import concourse.bacc as bacc
import concourse.tile as tile
from concourse import mybir
from concourse.bass import ts
from concourse.bass_utils import run_bass_kernel_spmd
from concourse.masks import make_identity

F32 = mybir.dt.float32
F16 = mybir.dt.float16
F8 = mybir.dt.float8e4
BF16 = mybir.dt.bfloat16
AF = mybir.ActivationFunctionType
ALU = mybir.AluOpType

B, H, S = 2, 16, 2048
CTX = 1024
VD = 288          # valence dim
FFN = 512
EPSILON = 0.066
LN_EPS = 1e-5

N_CORES = 8
HEADS_PER_CORE = H * B // N_CORES          # 4
SHARD_ROWS = HEADS_PER_CORE * S            # 8192
N_TOK = B * S                              # 4096 (global, for counts)
S_TILES = S // 128                         # 16 token tiles per core's batch
TOK_TILES = N_TOK // 128                   # 32 global token tiles
CTX_CHUNKS = CTX // 128                    # 8
ROWS_PER_CHUNK = 512                       # score rows per DMA chunk
N_CHUNKS = SHARD_ROWS // ROWS_PER_CHUNK    # 16
SUB_TILES = ROWS_PER_CHUNK // 128          # 4
STREAM_BUFS = 8


def _chunks(n, c=128):
    out = []
    o = 0
    while o < n:
        out.append((o, min(c, n - o)))
        o += c
    return out


def build_nc(mod=True, stream=True, mod_rep=1, stream_rep=1):
    nc = bacc.Bacc("TRN2", target_bir_lowering=False, debug=False,
                   num_devices=N_CORES)

    dt = nc.dram_tensor
    scores = dt("scores", [SHARD_ROWS, S], F16, kind="ExternalInput")
    ids_loc = dt("ids_loc", [1, S], F16, kind="ExternalInput")
    ids_rem = dt("ids_rem", [1, S], F16, kind="ExternalInput")
    iota_cols = dt("iota_cols", [128, CTX_CHUNKS], F32, kind="ExternalInput")
    embT = dt("embT", [VD, CTX], BF16, kind="ExternalInput")
    pW1 = dt("pW1", [VD, 2 * VD], BF16, kind="ExternalInput")
    pb1 = dt("pb1", [128, 5], F32, kind="ExternalInput")
    pW2 = dt("pW2", [2 * VD, H], BF16, kind="ExternalInput")
    pb2 = dt("pb2", [H, 1], F32, kind="ExternalInput")
    fW1 = dt("fW1", [H + 1, FFN], BF16, kind="ExternalInput")
    fb1 = dt("fb1", [128, 4], F32, kind="ExternalInput")
    fW2 = dt("fW2", [FFN, FFN // 2], BF16, kind="ExternalInput")
    fb2 = dt("fb2", [128, 2], F32, kind="ExternalInput")
    fW3 = dt("fW3", [FFN // 2, 1], BF16, kind="ExternalInput")
    fb3 = dt("fb3", [1, 1], F32, kind="ExternalInput")
    lnvw = dt("lnvw", [128, 3], F32, kind="ExternalInput")   # VD cols chunked
    lnvb = dt("lnvb", [128, 3], F32, kind="ExternalInput")
    lncw = dt("lncw", [H + 1, 1], F32, kind="ExternalInput")
    lncb = dt("lncb", [H + 1, 1], F32, kind="ExternalInput")
    out = dt("out", [SHARD_ROWS, S], F16, kind="ExternalOutput")

    VD_CH = _chunks(VD)            # [(0,128),(128,128),(256,32)]
    VD2_CH = _chunks(2 * VD)       # 576 -> 5 chunks
    FFN_CH = _chunks(FFN)          # 512 -> 4
    FFNH_CH = _chunks(FFN // 2)    # 256 -> 2
    NSPLIT = [(0, 512), (512, 512)]   # vocab free-dim split

    with tile.TileContext(nc) as tc, ExitStack() as ctx:
        singles = ctx.enter_context(tc.tile_pool(name="singles", bufs=1))
        work = ctx.enter_context(tc.tile_pool(name="work", bufs=8))
        otpool = ctx.enter_context(tc.tile_pool(name="otpool", bufs=1))
        ps = ctx.enter_context(tc.tile_pool(name="ps", bufs=3, space="PSUM"))
        pcol = ctx.enter_context(tc.tile_pool(name="pcol", bufs=1, space="PSUM"))
        sc_pool = ctx.enter_context(tc.tile_pool(name="sc", bufs=STREAM_BUFS))

        # ---------- constants / weights into SBUF ----------
        _uid = [0]

        def load(shape, src, dtype=F32):
            _uid[0] += 1
            t = singles.tile(shape, dtype, tag=f"s{_uid[0]}", name=f"s{_uid[0]}")
            nc.sync.dma_start(out=t[:], in_=src)
            return t

        def stile(shape, dtype=F32):
            _uid[0] += 1
            return singles.tile(shape, dtype, tag=f"s{_uid[0]}", name=f"s{_uid[0]}")

        embT_sb = []
        for o, sz in VD_CH:
            _t = work.tile([sz, CTX], BF16, tag="work", name=f"embT{o}")
            nc.sync.dma_start(out=_t[:], in_=embT[o:o + sz, :])
            embT_sb.append(_t)
        lnvw_sb = load([128, 3], lnvw[:, :])
        lnvb_sb = load([128, 3], lnvb[:, :])
        iota_cols_sb = load([128, CTX_CHUNKS], iota_cols[:, :], F32)
        ids_loc_b = stile([128, S], F16)
        nc.sync.dma_start(out=ids_loc_b[:], in_=ids_loc.ap().to_broadcast((128, S)))
        ids_rem_b = stile([128, S], F16)
        nc.sync.dma_start(out=ids_rem_b[:], in_=ids_rem.ap().to_broadcast((128, S)))
        pW1_sb = [load([sz, 2 * VD], pW1[o:o + sz, :], BF16) for o, sz in VD_CH]
        pW2_sb = [load([sz, H], pW2[o:o + sz, :], BF16) for o, sz in VD2_CH]
        fW1_sb = load([H + 1, FFN], fW1[:, :], BF16)
        fW2_sb = [load([sz, FFN // 2], fW2[o:o + sz, :], BF16) for o, sz in FFN_CH]
        fW3_sb = [load([sz, 1], fW3[o:o + sz, :], BF16) for o, sz in FFNH_CH]
        pb1_sb = load([128, 5], pb1[:, :])
        pb2_sb = load([H, 1], pb2[:, :])
        fb1_sb = load([128, 4], fb1[:, :])
        fb2_sb = load([128, 2], fb2[:, :])
        fb3_sb = load([1, 1], fb3[:, :])
        lncw_sb = load([H + 1, 1], lncw[:, :])
        lncb_sb = load([H + 1, 1], lncb[:, :])

        ones_col = stile([128, 1], BF16)
        nc.vector.memset(ones_col[:], 1.0)
        ones_row = stile([1, 128], BF16)
        nc.vector.memset(ones_row[:], 1.0)
        identity = stile([128, 128], F32)
        make_identity(nc, identity[:])

        eps_sb = stile([128, 1], F32)
        nc.vector.memset(eps_sb[:], LN_EPS)
        modv16 = stile([128, CTX_CHUNKS], F16)
        scales_sb = stile([128, S_TILES], F32)

        if not mod:
            nc.vector.memset(scales_sb[:], 1.0)
        for _mrep in range(mod_rep if mod else 0):
            # ---------- LN_v of emb table, transposed layout ----------
            # sum / sumsq rows over the 288 feature partitions via ones-lhsT matmul
            sum_ps = ps.tile([1, CTX], F32, tag="ps")
            sumsq_ps = ps.tile([1, CTX], F32, tag="ps")
            sq_t = []
            for k, (o, sz) in enumerate(VD_CH):
                sq = work.tile([sz, CTX], BF16, tag="work")
                nc.scalar.activation(out=sq[:], in_=embT_sb[k][:], func=AF.Square)
                sq_t.append(sq)
            for n0, nsz in NSPLIT:
                for k, (o, sz) in enumerate(VD_CH):
                    nc.tensor.matmul(sum_ps[:1, n0:n0 + nsz], ones_col[:sz, :],
                                     embT_sb[k][:, n0:n0 + nsz],
                                     start=(k == 0), stop=(k == len(VD_CH) - 1))
                for k, (o, sz) in enumerate(VD_CH):
                    nc.tensor.matmul(sumsq_ps[:1, n0:n0 + nsz], ones_col[:sz, :],
                                     sq_t[k][:, n0:n0 + nsz],
                                     start=(k == 0), stop=(k == len(VD_CH) - 1))


            # ---------- LN_v stats rows ----------
            m_row = work.tile([1, CTX], BF16, tag="work")
            nc.scalar.activation(out=m_row[:], in_=sum_ps[:1, :], func=AF.Copy,
                                 scale=1.0 / VD)
            msq_row = work.tile([1, CTX], F32, tag="work")
            nc.scalar.activation(out=msq_row[:], in_=sum_ps[:1, :], func=AF.Square,
                                 scale=1.0 / VD)
            varp = work.tile([1, CTX], F32, tag="work")
            nc.vector.tensor_scalar_mul(varp[:], sumsq_ps[:1, :], 1.0 / VD)
            nc.vector.tensor_tensor(out=varp[:], in0=varp[:], in1=msq_row[:],
                                    op=ALU.subtract)
            varp_bf = work.tile([1, CTX], BF16, tag="work")
            nc.scalar.activation(out=varp_bf[:], in_=varp[:],
                                 func=AF.Abs_reciprocal_sqrt, bias=eps_sb[:1, :])

            # broadcast mean / rstd rows across 128 partitions (bf16 PE matmul)
            mean_bc = ps.tile([128, CTX], F32, tag="ps")
            rstd_bc = ps.tile([128, CTX], F32, tag="ps")
            for n0, nsz in NSPLIT:
                nc.tensor.matmul(mean_bc[:, n0:n0 + nsz], ones_row[:1, :],
                                 m_row[:1, n0:n0 + nsz], start=True, stop=True)
                nc.tensor.matmul(rstd_bc[:, n0:n0 + nsz], ones_row[:1, :],
                                 varp_bf[:1, n0:n0 + nsz], start=True, stop=True)

            # ---------- normalized emb table E'T ----------
            e1t = []
            for k, (o, sz) in enumerate(VD_CH):
                e1 = work.tile([sz, CTX], F32, tag="work")
                nc.vector.tensor_tensor(out=e1[:], in0=embT_sb[k][:],
                                        in1=mean_bc[:sz, :], op=ALU.subtract)
                nc.vector.tensor_tensor(out=e1[:], in0=e1[:], in1=rstd_bc[:sz, :],
                                        op=ALU.mult)
                e1b = work.tile([sz, CTX], BF16, tag="work")
                nc.vector.tensor_scalar(e1b[:], e1[:], lnvw_sb[:sz, k:k + 1],
                                        lnvb_sb[:sz, k:k + 1],
                                        op0=ALU.mult, op1=ALU.add)
                e1t.append(e1b)

            # ---------- global token histogram (counts) ----------
            # One-hot tiles OT[c][p, t] = (ids[t] == c*128+p) serve double
            # duty: their fused accum_out column is this batch's histogram
            # contribution (sum over tokens), and the tiles feed the mod
            # gather matmuls at the end of the pipeline.  The remote batch
            # contributes via accum-only one-hots into a reused scratch tile.
            acc_loc = stile([128, CTX_CHUNKS], F32)
            acc_rem = stile([128, CTX_CHUNKS], F32)
            ot_sb = []
            for c in range(CTX_CHUNKS):
                _o = otpool.tile([128, S], F8, tag=f"ot{c}", name=f"ot{c}")
                nc.vector.tensor_scalar(_o[:], ids_loc_b[:],
                                        iota_cols_sb[:, c:c + 1], 0.0,
                                        op0=ALU.is_equal, op1=ALU.add,
                                        accum_out=acc_loc[:, c:c + 1])
                ot_sb.append(_o)
            for c in range(CTX_CHUNKS):
                otr = otpool.tile([128, S], F8, tag="otr", name=f"otr{c}")
                nc.vector.tensor_scalar(otr[:], ids_rem_b[:],
                                        iota_cols_sb[:, c:c + 1], 0.0,
                                        op0=ALU.is_equal, op1=ALU.add,
                                        accum_out=acc_rem[:, c:c + 1])
            counts_cs = stile([128, CTX_CHUNKS], F32)
            nc.vector.tensor_tensor(out=counts_cs[:], in0=acc_loc[:],
                                    in1=acc_rem[:], op=ALU.add)

            # ---------- layer A (gelu(E' @ pW1 + pb1)) fused with ----------
            # ---------- layer B (tanh(H1 @ pW2 + pb2)) accumulation ----------
            combT = work.tile([H + 1, CTX], BF16, tag="work")
            val_ps = ps.tile([H, CTX], F32, tag="ps")
            for m, (mo, msz) in enumerate(VD2_CH):
                h1_ps = ps.tile([msz, CTX], F32, tag="ps")
                for n0, nsz in NSPLIT:
                    for k, (o, sz) in enumerate(VD_CH):
                        nc.tensor.matmul(h1_ps[:, n0:n0 + nsz],
                                         pW1_sb[k][:, mo:mo + msz],
                                         e1t[k][:, n0:n0 + nsz],
                                         start=(k == 0),
                                         stop=(k == len(VD_CH) - 1))
                h1 = work.tile([msz, CTX], BF16, tag="work")
                nc.scalar.activation(out=h1[:], in_=h1_ps[:], func=AF.Gelu,
                                     bias=pb1_sb[:msz, m:m + 1])
                for n0, nsz in NSPLIT:
                    nc.tensor.matmul(val_ps[:, n0:n0 + nsz], pW2_sb[m][:, :],
                                     h1[:, n0:n0 + nsz],
                                     start=(m == 0), stop=(m == len(VD2_CH) - 1))

            counts_ps = ps.tile([1, CTX], F32, tag="ps")
            for c in range(CTX_CHUNKS):
                nc.tensor.transpose(counts_ps[:1, ts(c, 128)],
                                    counts_cs[:, c:c + 1], identity[:, :])

            # occupancy row: log1p(counts).  comb rows are stored permuted as
            # [valence(16 rows), occ] (host permutes fW1 / ln_c to match) because
            # SBUF partition-offset writes must be 32-aligned -- the occ row lands
            # in partition 16 via an SBUF->SBUF DMA instead.
            occ_sb = work.tile([1, CTX], BF16, tag="work")
            nc.scalar.activation(out=occ_sb[:], in_=counts_ps[:1, :],
                                 func=AF.Ln, bias=1.0)
            nc.sync.dma_start(out=combT[H:H + 1, :], in_=occ_sb[:1, :])
            nc.scalar.activation(out=combT[0:H, :], in_=val_ps[:], func=AF.Tanh,
                                 bias=pb2_sb[:H, 0:1])

            # ---------- LN_c over the 17 rows ----------
            sum17 = ps.tile([1, CTX], F32, tag="ps")
            sumsq17 = ps.tile([1, CTX], F32, tag="ps")
            sq17 = work.tile([H + 1, CTX], BF16, tag="work")
            nc.scalar.activation(out=sq17[:], in_=combT[:], func=AF.Square)
            for n0, nsz in NSPLIT:
                nc.tensor.matmul(sum17[:1, n0:n0 + nsz], ones_col[:H + 1, :],
                                 combT[:, n0:n0 + nsz], start=True, stop=True)
                nc.tensor.matmul(sumsq17[:1, n0:n0 + nsz], ones_col[:H + 1, :],
                                 sq17[:, n0:n0 + nsz], start=True, stop=True)
            m17 = work.tile([1, CTX], BF16, tag="work")
            nc.scalar.activation(out=m17[:], in_=sum17[:1, :], func=AF.Copy,
                                 scale=1.0 / (H + 1))
            msq17 = work.tile([1, CTX], F32, tag="work")
            nc.scalar.activation(out=msq17[:], in_=sum17[:1, :], func=AF.Square,
                                 scale=1.0 / (H + 1))
            varp17 = work.tile([1, CTX], F32, tag="work")
            nc.vector.tensor_scalar_mul(varp17[:], sumsq17[:1, :], 1.0 / (H + 1))
            nc.vector.tensor_tensor(out=varp17[:], in0=varp17[:], in1=msq17[:],
                                    op=ALU.subtract)
            varp17_bf = work.tile([1, CTX], BF16, tag="work")
            nc.scalar.activation(out=varp17_bf[:], in_=varp17[:],
                                 func=AF.Abs_reciprocal_sqrt, bias=eps_sb[:1, :])
            mean_bc17 = ps.tile([H + 1, CTX], F32, tag="ps")
            rstd_bc17 = ps.tile([H + 1, CTX], F32, tag="ps")
            for n0, nsz in NSPLIT:
                nc.tensor.matmul(mean_bc17[:, n0:n0 + nsz], ones_row[:1, :H + 1],
                                 m17[:1, n0:n0 + nsz], start=True, stop=True)
                nc.tensor.matmul(rstd_bc17[:, n0:n0 + nsz], ones_row[:1, :H + 1],
                                 varp17_bf[:1, n0:n0 + nsz], start=True, stop=True)
            comb2f = work.tile([H + 1, CTX], F32, tag="work")
            nc.vector.tensor_tensor(out=comb2f[:], in0=combT[:], in1=mean_bc17[:],
                                    op=ALU.subtract)
            nc.vector.tensor_tensor(out=comb2f[:], in0=comb2f[:],
                                    in1=rstd_bc17[:], op=ALU.mult)
            comb2 = work.tile([H + 1, CTX], BF16, tag="work")
            nc.vector.tensor_scalar(comb2[:], comb2f[:], lncw_sb[:, 0:1],
                                    lncb_sb[:, 0:1], op0=ALU.mult, op1=ALU.add)

            # ---------- layers D, E, F ----------
            h2t = []
            for m, (mo, msz) in enumerate(FFN_CH):
                h2_ps = ps.tile([msz, CTX], F32, tag="ps")
                for n0, nsz in NSPLIT:
                    nc.tensor.matmul(h2_ps[:, n0:n0 + nsz],
                                     fW1_sb[:, mo:mo + msz],
                                     comb2[:, n0:n0 + nsz], start=True, stop=True)
                h2 = work.tile([msz, CTX], BF16, tag="work")
                nc.scalar.activation(out=h2[:], in_=h2_ps[:], func=AF.Gelu,
                                     bias=fb1_sb[:msz, m:m + 1])
                h2t.append(h2)
            h3t = []
            for m, (mo, msz) in enumerate(FFNH_CH):
                h3_ps = ps.tile([msz, CTX], F32, tag="ps")
                for n0, nsz in NSPLIT:
                    for k, (o, sz) in enumerate(FFN_CH):
                        nc.tensor.matmul(h3_ps[:, n0:n0 + nsz],
                                         fW2_sb[k][:, mo:mo + msz],
                                         h2t[k][:, n0:n0 + nsz],
                                         start=(k == 0),
                                         stop=(k == len(FFN_CH) - 1))
                h3 = work.tile([msz, CTX], BF16, tag="work")
                nc.scalar.activation(out=h3[:], in_=h3_ps[:], func=AF.Gelu,
                                     bias=fb2_sb[:msz, m:m + 1])
                h3t.append(h3)
            mod_ps = ps.tile([1, CTX], F32, tag="ps")
            for n0, nsz in NSPLIT:
                for k, (o, sz) in enumerate(FFNH_CH):
                    nc.tensor.matmul(mod_ps[:1, n0:n0 + nsz], fW3_sb[k][:, :],
                                     h3t[k][:, n0:n0 + nsz],
                                     start=(k == 0), stop=(k == len(FFNH_CH) - 1))
            mod_row = work.tile([1, CTX], F32, tag="work")
            nc.scalar.activation(out=mod_row[:], in_=mod_ps[:1, :], func=AF.Tanh,
                                 bias=fb3_sb[:1, 0:1])

            # mod row -> per-chunk columns (PE transpose of [1,128] slices)
            modc_ps = pcol.tile([128, CTX_CHUNKS], F32, tag="pc")
            for c in range(CTX_CHUNKS):
                nc.tensor.transpose(modc_ps[:, c:c + 1],
                                    mod_row[:1, ts(c, 128)], identity[:1, :1])
            nc.vector.tensor_copy(modv16[:], modc_ps[:])   # cast f32 -> f16

            # ---------- gather per-token mod via one-hot matmuls ----------
            # mod_tok row = modv.T @ OT  (contract over ctx).  Done as two
            # [1, 1024] psum rows; each 512-slice is one psum bank whose
            # 8-matmul accumulation group completes before the next group's
            # start=True clears the bank's has_written bits.
            mtok_sb = singles.tile([1, S], F32, tag="mtok", name=f"mtok{_mrep}")
            for half in range(2):
                row_ps = ps.tile([1, 1024], F32, tag="ps", name=f"grow{half}")
                for n0 in (0, 512):
                    for c in range(CTX_CHUNKS):
                        nc.tensor.matmul(
                            row_ps[:1, n0:n0 + 512],
                            modv16[:, c:c + 1],
                            ot_sb[c][:, half * 1024 + n0:half * 1024 + n0 + 512],
                            start=(c == 0), stop=(c == CTX_CHUNKS - 1))
                nc.vector.tensor_copy(
                    mtok_sb[:1, half * 1024:(half + 1) * 1024], row_ps[:1, :])
            gath_ps = pcol.tile([128, S_TILES], F32, tag="pc")
            for t in range(S_TILES):
                nc.tensor.transpose(gath_ps[:, t:t + 1],
                                    mtok_sb[:1, ts(t, 128)], identity[:1, :1])
            nc.scalar.activation(out=scales_sb[:], in_=gath_ps[:], func=AF.Copy,
                                 bias=1.0, scale=EPSILON)

        if not stream:
            N_CH = 0
        else:
            N_CH = N_CHUNKS
        # ---------- the memory-bound scale of attention_scores ----------
        for j in range(N_CH * stream_rep):
            j = j % N_CHUNKS
            r0 = j * ROWS_PER_CHUNK
            src = scores[r0:r0 + ROWS_PER_CHUNK, :].rearrange(
                "(t p) k -> p t k", p=128)
            dst = out[r0:r0 + ROWS_PER_CHUNK, :].rearrange(
                "(t p) k -> p t k", p=128)
            sc = sc_pool.tile([128, SUB_TILES, S], F16, tag="sc")
            nc.sync.dma_start(out=sc[:], in_=src)
            for t in range(SUB_TILES):
                qt = (j % (S // ROWS_PER_CHUNK)) * SUB_TILES + t
                if t % 4 == 3:
                    nc.scalar.activation(out=sc[:, t, :], in_=sc[:, t, :],
                                         func=AF.Copy,
                                         scale=scales_sb[:, qt:qt + 1])
                else:
                    nc.vector.tensor_scalar_mul(sc[:, t, :], sc[:, t, :],
                                                scales_sb[:, qt:qt + 1])
            nc.sync.dma_start(out=dst, in_=sc[:])

    nc.finalize()
    return nc


_NC = None


def _get_nc():
    global _NC
    if _NC is None:
        _NC = build_nc()
    return _NC


def _cols(v, ncols):
    out = np.zeros((128, ncols), np.float32)
    v = v.reshape(-1)
    for k, (o, sz) in enumerate(_chunks(len(v))):
        out[:sz, k] = v[o:o + sz]
    return out


def build_in_maps(inputs):
    ids = np.asarray(inputs["input_ids"]).astype(np.int64)

    iota_cols = np.ascontiguousarray(
        np.arange(CTX, dtype=np.float32).reshape(CTX_CHUNKS, 128).T)

    f32 = lambda x: np.ascontiguousarray(np.asarray(x, dtype=np.float32))
    bf16 = lambda x: np.ascontiguousarray(
        np.asarray(x, dtype=np.float32).astype(ml_dtypes.bfloat16))
    embT = bf16(np.asarray(inputs["emb_W"]).T)
    lnvw = np.zeros((128, 3), np.float32)
    lnvb = np.zeros((128, 3), np.float32)
    wv = f32(inputs["ln_v_w"]).reshape(-1)
    bv = f32(inputs["ln_v_b"]).reshape(-1)
    for k, (o, sz) in enumerate(_chunks(VD)):
        lnvw[:sz, k] = wv[o:o + sz]
        lnvb[:sz, k] = bv[o:o + sz]

    common = {
        "iota_cols": iota_cols,
        "embT": embT,
        "pW1": bf16(inputs["pW1"]), "pb1": _cols(f32(inputs["pb1"]), 5),
        "pW2": bf16(inputs["pW2"]), "pb2": f32(inputs["pb2"]).reshape(-1, 1),
        "fW1": np.ascontiguousarray(np.roll(bf16(inputs["fW1"]), -1, axis=0)),
        "fb1": _cols(f32(inputs["fb1"]), 4),
        "fW2": bf16(inputs["fW2"]), "fb2": _cols(f32(inputs["fb2"]), 2),
        "fW3": bf16(inputs["fW3"]), "fb3": f32(inputs["fb3"]).reshape(1, -1),
        "lnvw": lnvw, "lnvb": lnvb,
        "lncw": np.roll(f32(inputs["ln_c_w"]), -1).reshape(-1, 1),
        "lncb": np.roll(f32(inputs["ln_c_b"]), -1).reshape(-1, 1),
    }

    scores = np.asarray(inputs["attention_scores"])
    scores_flat = scores.reshape(B * H, S, S)
    in_maps = []
    for i in range(N_CORES):
        b = i // (N_CORES // B)
        shard = np.ascontiguousarray(
            scores_flat[i * HEADS_PER_CORE:(i + 1) * HEADS_PER_CORE]
        ).reshape(SHARD_ROWS, S).astype(np.float16)
        m = dict(common)
        m["scores"] = shard
        m["ids_loc"] = ids[b].astype(np.float16).reshape(1, S)
        m["ids_rem"] = ids[1 - b].astype(np.float16).reshape(1, S)
        in_maps.append(m)
    return in_maps


def _run(inputs, **spmd_kwargs):
    in_maps = build_in_maps(inputs)
    nc = _get_nc()
    res = run_bass_kernel_spmd(nc, in_maps, core_ids=list(range(N_CORES)),
                               **spmd_kwargs)
    shards = [res.results[i]["out"] for i in range(N_CORES)]
    out = np.concatenate(shards, axis=0).reshape(B, H, S, S).astype(np.float32)
    return out, res


def kernel(**inputs) -> np.ndarray:
    return _run(inputs)[0]


if __name__ == "__main__":
    rng = np.random.default_rng(0)
    inputs = {
        "attention_scores": rng.standard_normal((B, H, S, S), dtype=np.float32),
        "input_ids": rng.integers(0, CTX, size=(B, S)),
        "emb_W": rng.standard_normal((CTX, VD), dtype=np.float32) * 0.05,
        "ln_v_w": np.ones(VD, np.float32), "ln_v_b": np.zeros(VD, np.float32),
        "pW1": rng.standard_normal((VD, 2 * VD), dtype=np.float32) * 0.05,
        "pb1": rng.standard_normal(2 * VD, dtype=np.float32) * 0.05,
        "pW2": rng.standard_normal((576, H), dtype=np.float32) * 0.04,
        "pb2": rng.standard_normal(H, dtype=np.float32) * 0.04,
        "ln_c_w": np.ones(H + 1, np.float32), "ln_c_b": np.zeros(H + 1, np.float32),
        "fW1": rng.standard_normal((H + 1, FFN), dtype=np.float32) * 0.2,
        "fb1": rng.standard_normal(FFN, dtype=np.float32) * 0.2,
        "fW2": rng.standard_normal((FFN, FFN // 2), dtype=np.float32) * 0.04,
        "fb2": rng.standard_normal(FFN // 2, dtype=np.float32) * 0.04,
        "fW3": rng.standard_normal((FFN // 2, 1), dtype=np.float32) * 0.06,
        "fb3": rng.standard_normal(1, dtype=np.float32) * 0.06,
    }
    out = kernel(**inputs)
    print("kernel output", out.shape, out.dtype, float(np.abs(out).mean()))


# revision 20
# speedup vs baseline: 1.9519x; 1.0275x over previous
"""Trainium2 Bass kernel for nn_MemoryWeightedAttention.

out[b,h,q,k] = attention_scores[b,h,q,k] * (1 + 0.066 * mod[b,q])

where mod[b,q] is a small LN/MLP pipeline applied to
(log1p(global_count[id]), tanh-MLP(LN(emb[id]))) -- i.e. mod depends ONLY on
the token id at (b,q) plus a global histogram of input_ids.  So we compute the
full pipeline once over the 1024-entry vocabulary table (in transposed layout:
features on partitions, vocab on the free dim -> no transposes between
layers), then gather per-token scales with one-hot matmuls, and stream the
attention_scores tensor (as fp16) through a per-partition scalar multiply.

Sharding: 8 cores, core i handles batch i//4, heads 4*(i%4) .. 4*(i%4)+4
(a contiguous [8192, 2048] row-slice of the flattened [65536, 2048] scores).
The tiny table pipeline is replicated on every core (no collectives).

Precision: the scores stream is fp16 (input quantization ~5e-4 rel) and the
table pipeline runs in bf16 on the PE (4x faster than fp32); the resulting
scale factor (1 + 0.066*tanh(...)) error is ~1e-4.  Total mean rel err
~5e-4, far under the 2e-2 gate.
"""

import sys

for _p in ("/opt/trn_rl_repo",):
    if _p not in sys.path:
        sys.path.insert(0, _p)

from contextlib import ExitStack

import numpy as np
import ml_dtypes

import concourse.bacc as bacc
import concourse.tile as tile
from concourse import mybir
from concourse.bass import ts
from concourse.bass_utils import run_bass_kernel_spmd
from concourse.masks import make_identity

F32 = mybir.dt.float32
F16 = mybir.dt.float16
F8 = mybir.dt.float8e4
BF16 = mybir.dt.bfloat16
AF = mybir.ActivationFunctionType
ALU = mybir.AluOpType

B, H, S = 2, 16, 2048
CTX = 1024
VD = 288          # valence dim
FFN = 512
EPSILON = 0.066
LN_EPS = 1e-5

N_CORES = 8
HEADS_PER_CORE = H * B // N_CORES          # 4
SHARD_ROWS = HEADS_PER_CORE * S            # 8192
N_TOK = B * S                              # 4096 (global, for counts)
S_TILES = S // 128                         # 16 token tiles per core's batch
TOK_TILES = N_TOK // 128                   # 32 global token tiles
CTX_CHUNKS = CTX // 128                    # 8
ROWS_PER_CHUNK = 512                       # score rows per DMA chunk
N_CHUNKS = SHARD_ROWS // ROWS_PER_CHUNK    # 16
SUB_TILES = ROWS_PER_CHUNK // 128          # 4
STREAM_BUFS = 8


def _chunks(n, c=128):
    out = []
    o = 0
    while o < n:
        out.append((o, min(c, n - o)))
        o += c
    return out


def build_nc(mod=True, stream=True, mod_rep=1, stream_rep=1):
    nc = bacc.Bacc("TRN2", target_bir_lowering=False, debug=False,
                   num_devices=N_CORES)

    dt = nc.dram_tensor
    scores = dt("scores", [SHARD_ROWS, S], F16, kind="ExternalInput")
    ids_loc = dt("ids_loc", [1, S], F16, kind="ExternalInput")
    ids_rem = dt("ids_rem", [1, S], F16, kind="ExternalInput")
    iota_cols = dt("iota_cols", [128, CTX_CHUNKS], F32, kind="ExternalInput")
    embT = dt("embT", [VD, CTX], BF16, kind="ExternalInput")
    pW1 = dt("pW1", [VD, 2 * VD], BF16, kind="ExternalInput")
    pb1 = dt("pb1", [128, 5], F32, kind="ExternalInput")
    pW2 = dt("pW2", [2 * VD, H], BF16, kind="ExternalInput")
    pb2 = dt("pb2", [H, 1], F32, kind="ExternalInput")
    fW1 = dt("fW1", [H + 1, FFN], BF16, kind="ExternalInput")
    fb1 = dt("fb1", [128, 4], F32, kind="ExternalInput")
    fW2 = dt("fW2", [FFN, FFN // 2], BF16, kind="ExternalInput")
    fb2 = dt("fb2", [128, 2], F32, kind="ExternalInput")
    fW3 = dt("fW3", [FFN // 2, 1], BF16, kind="ExternalInput")
    fb3 = dt("fb3", [1, 1], F32, kind="ExternalInput")
    lnvw = dt("lnvw", [128, 3], F32, kind="ExternalInput")   # VD cols chunked
    lnvb = dt("lnvb", [128, 3], F32, kind="ExternalInput")
    lncw = dt("lncw", [H + 1, 1], F32, kind="ExternalInput")
    lncb = dt("lncb", [H + 1, 1], F32, kind="ExternalInput")
    out = dt("out", [SHARD_ROWS, S], F16, kind="ExternalOutput")

    VD_CH = _chunks(VD)            # [(0,128),(128,128),(256,32)]
    VD2_CH = _chunks(2 * VD)       # 576 -> 5 chunks
    FFN_CH = _chunks(FFN)          # 512 -> 4
    FFNH_CH = _chunks(FFN // 2)    # 256 -> 2
    NSPLIT = [(0, 512), (512, 512)]   # vocab free-dim split

    with tile.TileContext(nc) as tc, ExitStack() as ctx:
        singles = ctx.enter_context(tc.tile_pool(name="singles", bufs=1))
        work = ctx.enter_context(tc.tile_pool(name="work", bufs=8))
        otpool = ctx.enter_context(tc.tile_pool(name="otpool", bufs=1))
        ps = ctx.enter_context(tc.tile_pool(name="ps", bufs=3, space="PSUM"))
        pcol = ctx.enter_context(tc.tile_pool(name="pcol", bufs=1, space="PSUM"))
        sc_pool = ctx.enter_context(tc.tile_pool(name="sc", bufs=STREAM_BUFS))

        # ---------- constants / weights into SBUF ----------
        _uid = [0]

        def load(shape, src, dtype=F32):
            _uid[0] += 1
            t = singles.tile(shape, dtype, tag=f"s{_uid[0]}", name=f"s{_uid[0]}")
            nc.sync.dma_start(out=t[:], in_=src)
            return t

        def stile(shape, dtype=F32):
            _uid[0] += 1
            return singles.tile(shape, dtype, tag=f"s{_uid[0]}", name=f"s{_uid[0]}")

        embT_sb = []
        for o, sz in VD_CH:
            _t = work.tile([sz, CTX], BF16, tag="work", name=f"embT{o}")
            nc.sync.dma_start(out=_t[:], in_=embT[o:o + sz, :])
            embT_sb.append(_t)
        lnvw_sb = load([128, 3], lnvw[:, :])
        lnvb_sb = load([128, 3], lnvb[:, :])
        iota_cols_sb = load([128, CTX_CHUNKS], iota_cols[:, :], F32)
        ids_loc_b = stile([128, S], F16)
        nc.sync.dma_start(out=ids_loc_b[:], in_=ids_loc.ap().to_broadcast((128, S)))
        ids_rem_b = stile([128, S], F16)
        nc.sync.dma_start(out=ids_rem_b[:], in_=ids_rem.ap().to_broadcast((128, S)))
        pW1_sb = [load([sz, 2 * VD], pW1[o:o + sz, :], BF16) for o, sz in VD_CH]
        pW2_sb = [load([sz, H], pW2[o:o + sz, :], BF16) for o, sz in VD2_CH]
        fW1_sb = load([H + 1, FFN], fW1[:, :], BF16)
        fW2_sb = [load([sz, FFN // 2], fW2[o:o + sz, :], BF16) for o, sz in FFN_CH]
        fW3_sb = [load([sz, 1], fW3[o:o + sz, :], BF16) for o, sz in FFNH_CH]
        pb1_sb = load([128, 5], pb1[:, :])
        pb2_sb = load([H, 1], pb2[:, :])
        fb1_sb = load([128, 4], fb1[:, :])
        fb2_sb = load([128, 2], fb2[:, :])
        fb3_sb = load([1, 1], fb3[:, :])
        lncw_sb = load([H + 1, 1], lncw[:, :])
        lncb_sb = load([H + 1, 1], lncb[:, :])

        ones_col = stile([128, 1], BF16)
        nc.vector.memset(ones_col[:], 1.0)
        ones_row = stile([1, 128], BF16)
        nc.vector.memset(ones_row[:], 1.0)
        identity = stile([128, 128], F32)
        make_identity(nc, identity[:])

        eps_sb = stile([128, 1], F32)
        nc.vector.memset(eps_sb[:], LN_EPS)
        modv16 = stile([128, CTX_CHUNKS], F16)
        scales_sb = stile([128, S_TILES], F32)

        if not mod:
            nc.vector.memset(scales_sb[:], 1.0)
        for _mrep in range(mod_rep if mod else 0):
            # ---------- LN_v of emb table, transposed layout ----------
            # sum / sumsq rows over the 288 feature partitions via ones-lhsT matmul
            sum_ps = ps.tile([1, CTX], F32, tag="ps")
            sumsq_ps = ps.tile([1, CTX], F32, tag="ps")
            sq_t = []
            for k, (o, sz) in enumerate(VD_CH):
                sq = work.tile([sz, CTX], BF16, tag="work")
                nc.scalar.activation(out=sq[:], in_=embT_sb[k][:], func=AF.Square)
                sq_t.append(sq)
            for n0, nsz in NSPLIT:
                for k, (o, sz) in enumerate(VD_CH):
                    nc.tensor.matmul(sum_ps[:1, n0:n0 + nsz], ones_col[:sz, :],
                                     embT_sb[k][:, n0:n0 + nsz],
                                     start=(k == 0), stop=(k == len(VD_CH) - 1))
                for k, (o, sz) in enumerate(VD_CH):
                    nc.tensor.matmul(sumsq_ps[:1, n0:n0 + nsz], ones_col[:sz, :],
                                     sq_t[k][:, n0:n0 + nsz],
                                     start=(k == 0), stop=(k == len(VD_CH) - 1))


            # ---------- LN_v stats rows ----------
            m_row = work.tile([1, CTX], BF16, tag="work")
            nc.scalar.activation(out=m_row[:], in_=sum_ps[:1, :], func=AF.Copy,
                                 scale=1.0 / VD)
            msq_row = work.tile([1, CTX], F32, tag="work")
            nc.scalar.activation(out=msq_row[:], in_=sum_ps[:1, :], func=AF.Square,
                                 scale=1.0 / VD)
            varp = work.tile([1, CTX], F32, tag="work")
            nc.vector.tensor_scalar_mul(varp[:], sumsq_ps[:1, :], 1.0 / VD)
            nc.vector.tensor_tensor(out=varp[:], in0=varp[:], in1=msq_row[:],
                                    op=ALU.subtract)
            varp_ln = work.tile([1, CTX], F32, tag="work")
            nc.scalar.activation(out=varp_ln[:], in_=varp[:], func=AF.Ln,
                                 bias=eps_sb[:1, :])
            varp_bf = work.tile([1, CTX], BF16, tag="work")
            nc.scalar.activation(out=varp_bf[:], in_=varp_ln[:], func=AF.Exp,
                                 scale=-0.5)

            # broadcast mean / rstd rows across 128 partitions (bf16 PE matmul)
            mean_bc = ps.tile([128, CTX], F32, tag="ps")
            rstd_bc = ps.tile([128, CTX], F32, tag="ps")
            for n0, nsz in NSPLIT:
                nc.tensor.matmul(mean_bc[:, n0:n0 + nsz], ones_row[:1, :],
                                 m_row[:1, n0:n0 + nsz], start=True, stop=True)
                nc.tensor.matmul(rstd_bc[:, n0:n0 + nsz], ones_row[:1, :],
                                 varp_bf[:1, n0:n0 + nsz], start=True, stop=True)

            # ---------- normalized emb table E'T ----------
            e1t = []
            for k, (o, sz) in enumerate(VD_CH):
                e1 = work.tile([sz, CTX], F32, tag="work")
                nc.vector.tensor_tensor(out=e1[:], in0=embT_sb[k][:],
                                        in1=mean_bc[:sz, :], op=ALU.subtract)
                nc.vector.tensor_tensor(out=e1[:], in0=e1[:], in1=rstd_bc[:sz, :],
                                        op=ALU.mult)
                e1b = work.tile([sz, CTX], BF16, tag="work")
                nc.vector.tensor_scalar(e1b[:], e1[:], lnvw_sb[:sz, k:k + 1],
                                        lnvb_sb[:sz, k:k + 1],
                                        op0=ALU.mult, op1=ALU.add)
                e1t.append(e1b)

            # ---------- global token histogram (counts) ----------
            # One-hot tiles OT[c][p, t] = (ids[t] == c*128+p) serve double
            # duty: their fused accum_out column is this batch's histogram
            # contribution (sum over tokens), and the tiles feed the mod
            # gather matmuls at the end of the pipeline.  The remote batch
            # contributes via accum-only one-hots into a reused scratch tile.
            acc_loc = stile([128, CTX_CHUNKS], F32)
            acc_rem = stile([128, CTX_CHUNKS], F32)
            ot_sb = []
            for c in range(CTX_CHUNKS):
                _o = otpool.tile([128, S], F8, tag=f"ot{c}", name=f"ot{c}")
                nc.vector.tensor_scalar(_o[:], ids_loc_b[:],
                                        iota_cols_sb[:, c:c + 1], 0.0,
                                        op0=ALU.is_equal, op1=ALU.add,
                                        accum_out=acc_loc[:, c:c + 1])
                ot_sb.append(_o)
            for c in range(CTX_CHUNKS):
                otr = otpool.tile([128, S], F8, tag="otr", name=f"otr{c}")
                nc.vector.tensor_scalar(otr[:], ids_rem_b[:],
                                        iota_cols_sb[:, c:c + 1], 0.0,
                                        op0=ALU.is_equal, op1=ALU.add,
                                        accum_out=acc_rem[:, c:c + 1])
            counts_cs = stile([128, CTX_CHUNKS], F32)
            nc.vector.tensor_tensor(out=counts_cs[:], in0=acc_loc[:],
                                    in1=acc_rem[:], op=ALU.add)

            # ---------- layer A (gelu(E' @ pW1 + pb1)) fused with ----------
            # ---------- layer B (tanh(H1 @ pW2 + pb2)) accumulation ----------
            combT = work.tile([H + 1, CTX], BF16, tag="work")
            val_ps = ps.tile([H, CTX], F32, tag="ps")
            for m, (mo, msz) in enumerate(VD2_CH):
                h1_ps = ps.tile([msz, CTX], F32, tag="ps")
                for n0, nsz in NSPLIT:
                    for k, (o, sz) in enumerate(VD_CH):
                        nc.tensor.matmul(h1_ps[:, n0:n0 + nsz],
                                         pW1_sb[k][:, mo:mo + msz],
                                         e1t[k][:, n0:n0 + nsz],
                                         start=(k == 0),
                                         stop=(k == len(VD_CH) - 1))
                h1 = work.tile([msz, CTX], BF16, tag="work")
                nc.scalar.activation(out=h1[:], in_=h1_ps[:], func=AF.Gelu,
                                     bias=pb1_sb[:msz, m:m + 1])
                for n0, nsz in NSPLIT:
                    nc.tensor.matmul(val_ps[:, n0:n0 + nsz], pW2_sb[m][:, :],
                                     h1[:, n0:n0 + nsz],
                                     start=(m == 0), stop=(m == len(VD2_CH) - 1))

            counts_ps = ps.tile([1, CTX], F32, tag="ps")
            for c in range(CTX_CHUNKS):
                nc.tensor.transpose(counts_ps[:1, ts(c, 128)],
                                    counts_cs[:, c:c + 1], identity[:, :])

            # occupancy row: log1p(counts).  comb rows are stored permuted as
            # [valence(16 rows), occ] (host permutes fW1 / ln_c to match) because
            # SBUF partition-offset writes must be 32-aligned -- the occ row lands
            # in partition 16 via an SBUF->SBUF DMA instead.
            occ_sb = work.tile([1, CTX], BF16, tag="work")
            nc.scalar.activation(out=occ_sb[:], in_=counts_ps[:1, :],
                                 func=AF.Ln, bias=1.0)
            nc.sync.dma_start(out=combT[H:H + 1, :], in_=occ_sb[:1, :])
            nc.scalar.activation(out=combT[0:H, :], in_=val_ps[:], func=AF.Tanh,
                                 bias=pb2_sb[:H, 0:1])

            # ---------- LN_c over the 17 rows ----------
            sum17 = ps.tile([1, CTX], F32, tag="ps")
            sumsq17 = ps.tile([1, CTX], F32, tag="ps")
            sq17 = work.tile([H + 1, CTX], BF16, tag="work")
            nc.scalar.activation(out=sq17[:], in_=combT[:], func=AF.Square)
            for n0, nsz in NSPLIT:
                nc.tensor.matmul(sum17[:1, n0:n0 + nsz], ones_col[:H + 1, :],
                                 combT[:, n0:n0 + nsz], start=True, stop=True)
                nc.tensor.matmul(sumsq17[:1, n0:n0 + nsz], ones_col[:H + 1, :],
                                 sq17[:, n0:n0 + nsz], start=True, stop=True)
            m17 = work.tile([1, CTX], BF16, tag="work")
            nc.scalar.activation(out=m17[:], in_=sum17[:1, :], func=AF.Copy,
                                 scale=1.0 / (H + 1))
            msq17 = work.tile([1, CTX], F32, tag="work")
            nc.scalar.activation(out=msq17[:], in_=sum17[:1, :], func=AF.Square,
                                 scale=1.0 / (H + 1))
            varp17 = work.tile([1, CTX], F32, tag="work")
            nc.vector.tensor_scalar_mul(varp17[:], sumsq17[:1, :], 1.0 / (H + 1))
            nc.vector.tensor_tensor(out=varp17[:], in0=varp17[:], in1=msq17[:],
                                    op=ALU.subtract)
            varp17_ln = work.tile([1, CTX], F32, tag="work")
            nc.scalar.activation(out=varp17_ln[:], in_=varp17[:], func=AF.Ln,
                                 bias=eps_sb[:1, :])
            varp17_bf = work.tile([1, CTX], BF16, tag="work")
            nc.scalar.activation(out=varp17_bf[:], in_=varp17_ln[:], func=AF.Exp,
                                 scale=-0.5)
            mean_bc17 = ps.tile([H + 1, CTX], F32, tag="ps")
            rstd_bc17 = ps.tile([H + 1, CTX], F32, tag="ps")
            for n0, nsz in NSPLIT:
                nc.tensor.matmul(mean_bc17[:, n0:n0 + nsz], ones_row[:1, :H + 1],
                                 m17[:1, n0:n0 + nsz], start=True, stop=True)
                nc.tensor.matmul(rstd_bc17[:, n0:n0 + nsz], ones_row[:1, :H + 1],
                                 varp17_bf[:1, n0:n0 + nsz], start=True, stop=True)
            comb2f = work.tile([H + 1, CTX], F32, tag="work")
            nc.vector.tensor_tensor(out=comb2f[:], in0=combT[:], in1=mean_bc17[:],
                                    op=ALU.subtract)
            nc.vector.tensor_tensor(out=comb2f[:], in0=comb2f[:],
                                    in1=rstd_bc17[:], op=ALU.mult)
            comb2 = work.tile([H + 1, CTX], BF16, tag="work")
            nc.vector.tensor_scalar(comb2[:], comb2f[:], lncw_sb[:, 0:1],
                                    lncb_sb[:, 0:1], op0=ALU.mult, op1=ALU.add)

            # ---------- layers D, E, F ----------
            h2t = []
            for m, (mo, msz) in enumerate(FFN_CH):
                h2_ps = ps.tile([msz, CTX], F32, tag="ps")
                for n0, nsz in NSPLIT:
                    nc.tensor.matmul(h2_ps[:, n0:n0 + nsz],
                                     fW1_sb[:, mo:mo + msz],
                                     comb2[:, n0:n0 + nsz], start=True, stop=True)
                h2 = work.tile([msz, CTX], BF16, tag="work")
                nc.scalar.activation(out=h2[:], in_=h2_ps[:], func=AF.Gelu,
                                     bias=fb1_sb[:msz, m:m + 1])
                h2t.append(h2)
            h3t = []
            for m, (mo, msz) in enumerate(FFNH_CH):
                h3_ps = ps.tile([msz, CTX], F32, tag="ps")
                for n0, nsz in NSPLIT:
                    for k, (o, sz) in enumerate(FFN_CH):
                        nc.tensor.matmul(h3_ps[:, n0:n0 + nsz],
                                         fW2_sb[k][:, mo:mo + msz],
                                         h2t[k][:, n0:n0 + nsz],
                                         start=(k == 0),
                                         stop=(k == len(FFN_CH) - 1))
                h3 = work.tile([msz, CTX], BF16, tag="work")
                nc.scalar.activation(out=h3[:], in_=h3_ps[:], func=AF.Gelu,
                                     bias=fb2_sb[:msz, m:m + 1])
                h3t.append(h3)
            mod_ps = ps.tile([1, CTX], F32, tag="ps")
            for n0, nsz in NSPLIT:
                for k, (o, sz) in enumerate(FFNH_CH):
                    nc.tensor.matmul(mod_ps[:1, n0:n0 + nsz], fW3_sb[k][:, :],
                                     h3t[k][:, n0:n0 + nsz],
                                     start=(k == 0), stop=(k == len(FFNH_CH) - 1))
            mod_row = work.tile([1, CTX], F32, tag="work")
            nc.scalar.activation(out=mod_row[:], in_=mod_ps[:1, :], func=AF.Tanh,
                                 bias=fb3_sb[:1, 0:1])

            # mod row -> per-chunk columns (PE transpose of [1,128] slices)
            modc_ps = pcol.tile([128, CTX_CHUNKS], F32, tag="pc")
            for c in range(CTX_CHUNKS):
                nc.tensor.transpose(modc_ps[:, c:c + 1],
                                    mod_row[:1, ts(c, 128)], identity[:1, :1])
            nc.vector.tensor_copy(modv16[:], modc_ps[:])   # cast f32 -> f16

            # ---------- gather per-token mod via one-hot matmuls ----------
            # mod_tok row = modv.T @ OT  (contract over ctx).  Done as two
            # [1, 1024] psum rows; each 512-slice is one psum bank whose
            # 8-matmul accumulation group completes before the next group's
            # start=True clears the bank's has_written bits.
            mtok_sb = singles.tile([1, S], F32, tag="mtok", name=f"mtok{_mrep}")
            for half in range(2):
                row_ps = ps.tile([1, 1024], F32, tag="ps", name=f"grow{half}")
                for n0 in (0, 512):
                    for c in range(CTX_CHUNKS):
                        nc.tensor.matmul(
                            row_ps[:1, n0:n0 + 512],
                            modv16[:, c:c + 1],
                            ot_sb[c][:, half * 1024 + n0:half * 1024 + n0 + 512],
                            start=(c == 0), stop=(c == CTX_CHUNKS - 1))
                nc.vector.tensor_copy(
                    mtok_sb[:1, half * 1024:(half + 1) * 1024], row_ps[:1, :])
            gath_ps = pcol.tile([128, S_TILES], F32, tag="pc")
            for t in range(S_TILES):
                nc.tensor.transpose(gath_ps[:, t:t + 1],
                                    mtok_sb[:1, ts(t, 128)], identity[:1, :1])
            nc.scalar.activation(out=scales_sb[:], in_=gath_ps[:], func=AF.Copy,
                                 bias=1.0, scale=EPSILON)

        if not stream:
            N_CH = 0
        else:
            N_CH = N_CHUNKS
        # ---------- the memory-bound scale of attention_scores ----------
        for j in range(N_CH * stream_rep):
            j = j % N_CHUNKS
            r0 = j * ROWS_PER_CHUNK
            src = scores[r0:r0 + ROWS_PER_CHUNK, :].rearrange(
                "(t p) k -> p t k", p=128)
            dst = out[r0:r0 + ROWS_PER_CHUNK, :].rearrange(
                "(t p) k -> p t k", p=128)
            sc = sc_pool.tile([128, SUB_TILES, S], F16, tag="sc")
            nc.sync.dma_start(out=sc[:], in_=src)
            for t in range(SUB_TILES):
                qt = (j % (S // ROWS_PER_CHUNK)) * SUB_TILES + t
                if t % 4 == 3:
                    nc.scalar.activation(out=sc[:, t, :], in_=sc[:, t, :],
                                         func=AF.Copy,
                                         scale=scales_sb[:, qt:qt + 1])
                else:
                    nc.vector.tensor_scalar_mul(sc[:, t, :], sc[:, t, :],
                                                scales_sb[:, qt:qt + 1])
            nc.sync.dma_start(out=dst, in_=sc[:])

    nc.finalize()
    return nc


_NC = None


def _get_nc():
    global _NC
    if _NC is None:
        _NC = build_nc()
    return _NC


def _cols(v, ncols):
    out = np.zeros((128, ncols), np.float32)
    v = v.reshape(-1)
    for k, (o, sz) in enumerate(_chunks(len(v))):
        out[:sz, k] = v[o:o + sz]
    return out


def build_in_maps(inputs):
    ids = np.asarray(inputs["input_ids"]).astype(np.int64)

    iota_cols = np.ascontiguousarray(
        np.arange(CTX, dtype=np.float32).reshape(CTX_CHUNKS, 128).T)

    f32 = lambda x: np.ascontiguousarray(np.asarray(x, dtype=np.float32))
    bf16 = lambda x: np.ascontiguousarray(
        np.asarray(x, dtype=np.float32).astype(ml_dtypes.bfloat16))
    embT = bf16(np.asarray(inputs["emb_W"]).T)
    lnvw = np.zeros((128, 3), np.float32)
    lnvb = np.zeros((128, 3), np.float32)
    wv = f32(inputs["ln_v_w"]).reshape(-1)
    bv = f32(inputs["ln_v_b"]).reshape(-1)
    for k, (o, sz) in enumerate(_chunks(VD)):
        lnvw[:sz, k] = wv[o:o + sz]
        lnvb[:sz, k] = bv[o:o + sz]

    common = {
        "iota_cols": iota_cols,
        "embT": embT,
        "pW1": bf16(inputs["pW1"]), "pb1": _cols(f32(inputs["pb1"]), 5),
        "pW2": bf16(inputs["pW2"]), "pb2": f32(inputs["pb2"]).reshape(-1, 1),
        "fW1": np.ascontiguousarray(np.roll(bf16(inputs["fW1"]), -1, axis=0)),
        "fb1": _cols(f32(inputs["fb1"]), 4),
        "fW2": bf16(inputs["fW2"]), "fb2": _cols(f32(inputs["fb2"]), 2),
        "fW3": bf16(inputs["fW3"]), "fb3": f32(inputs["fb3"]).reshape(1, -1),
        "lnvw": lnvw, "lnvb": lnvb,
        "lncw": np.roll(f32(inputs["ln_c_w"]), -1).reshape(-1, 1),
        "lncb": np.roll(f32(inputs["ln_c_b"]), -1).reshape(-1, 1),
    }

    scores = np.asarray(inputs["attention_scores"])
    scores_flat = scores.reshape(B * H, S, S)
    in_maps = []
    for i in range(N_CORES):
        b = i // (N_CORES // B)
        shard = np.ascontiguousarray(
            scores_flat[i * HEADS_PER_CORE:(i + 1) * HEADS_PER_CORE]
        ).reshape(SHARD_ROWS, S).astype(np.float16)
        m = dict(common)
        m["scores"] = shard
        m["ids_loc"] = ids[b].astype(np.float16).reshape(1, S)
        m["ids_rem"] = ids[1 - b].astype(np.float16).reshape(1, S)
        in_maps.append(m)
    return in_maps


def _run(inputs, **spmd_kwargs):
    in_maps = build_in_maps(inputs)
    nc = _get_nc()
    res = run_bass_kernel_spmd(nc, in_maps, core_ids=list(range(N_CORES)),
                               **spmd_kwargs)
    shards = [res.results[i]["out"] for i in range(N_CORES)]
    out = np.concatenate(shards, axis=0).reshape(B, H, S, S).astype(np.float32)
    return out, res


def kernel(**inputs) -> np.ndarray:
    return _run(inputs)[0]


if __name__ == "__main__":
    rng = np.random.default_rng(0)
    inputs = {
        "attention_scores": rng.standard_normal((B, H, S, S), dtype=np.float32),
        "input_ids": rng.integers(0, CTX, size=(B, S)),
        "emb_W": rng.standard_normal((CTX, VD), dtype=np.float32) * 0.05,
        "ln_v_w": np.ones(VD, np.float32), "ln_v_b": np.zeros(VD, np.float32),
        "pW1": rng.standard_normal((VD, 2 * VD), dtype=np.float32) * 0.05,
        "pb1": rng.standard_normal(2 * VD, dtype=np.float32) * 0.05,
        "pW2": rng.standard_normal((576, H), dtype=np.float32) * 0.04,
        "pb2": rng.standard_normal(H, dtype=np.float32) * 0.04,
        "ln_c_w": np.ones(H + 1, np.float32), "ln_c_b": np.zeros(H + 1, np.float32),
        "fW1": rng.standard_normal((H + 1, FFN), dtype=np.float32) * 0.2,
        "fb1": rng.standard_normal(FFN, dtype=np.float32) * 0.2,
        "fW2": rng.standard_normal((FFN, FFN // 2), dtype=np.float32) * 0.04,
        "fb2": rng.standard_normal(FFN // 2, dtype=np.float32) * 0.04,
        "fW3": rng.standard_normal((FFN // 2, 1), dtype=np.float32) * 0.06,
        "fb3": rng.standard_normal(1, dtype=np.float32) * 0.06,
    }
    out = kernel(**inputs)
    print("kernel output", out.shape, out.dtype, float(np.abs(out).mean()))
